# revision 4
# baseline (speedup 1.0000x reference)
"""Trainium2 Bass kernel for the 2-layer GAT (nn_GATNet).

Strategy (destination-sharded, 8 NeuronCores):
- Nodes (and their incoming edges) are partitioned across cores in
  contiguous id ranges.  Each core processes its destinations in
  degree-sorted tiles of 128.
- A prepass matmul builds T1 = [h(64) | ls(8) | ld(8)] rows for all
  nodes (replicated on every core).
- Per-edge source rows are fetched with the GPSIMD multi-index
  dma_gather (int16 indices -> the id space is split at 32768 into an
  A range (base 0) and B range (base 32768); rectangle padding points
  at row 0 of the range and is cancelled with host-provided pad
  counts).
- Segment softmax is computed densely per tile (dst on partitions,
  edge slots on the free dim); self-loops are applied densely from an
  own-row gather.
- Layer-2 table T2 = [h2(16) | ls2 | ld2] is built in processing
  order, staged to DRAM and AllGathered across the 8 cores; layer 2
  then repeats the gather/softmax with 17-column rows.
- log_softmax is computed per tile; the host undoes the processing
  permutation.
"""
import time

import numpy as np
import ml_dtypes

import concourse.tile as tile
import concourse.mybir as mybir
from concourse import library_config
from concourse.bass import Bass, exact_div
from concourse.bass_utils import run_bass_kernel_spmd

BF16 = mybir.dt.bfloat16
F32 = mybir.dt.float32
I16 = mybir.dt.int16

P = 128

# ---------------------------------------------------------------- wait fix
_ctr = [0]


def _legalize_waits(nc):
    """This walrus build rejects instructions with >1 semaphore wait.
    Split multi-wait instructions into single-wait NoOp carriers."""
    for fn in nc.m.functions:
        for bb in fn.blocks:
            insts = list(bb.instructions)
            out = []
            changed = False
            for inst in insts:
                si = inst.sync_info
                if si is not None and si.on_wait is not None and len(si.on_wait) > 1:
                    waits = list(si.on_wait)
                    ups = list(si.on_update) if si.on_update is not None else []
                    for w in waits[:-1]:
                        _ctr[0] += 1
                        nop = mybir.InstNoOp(name=f"waitnop-{_ctr[0]}")
                        nop.engine = inst.engine
                        nop.sync_info = mybir.SyncInfo(on_wait=[w], on_update=[])
                        out.append(nop)
                    inst.sync_info = mybir.SyncInfo(on_wait=[waits[-1]], on_update=ups)
                    changed = True
                out.append(inst)
            if changed:
                bb.instructions = out


def _dma_gather(gp, out_ap, in_ap, idxs_ap, num_idxs, reg, elem_size):
    """dma_gather without the (transpose-only) elem%256 assert.
    in_ap row stride must be a multiple of 256 bytes."""
    elem_step = in_ap.ap[0][0]
    stride_bytes_256 = exact_div(elem_step * mybir.dt.size(in_ap.dtype), 256)
    _in_ap = gp.lower_ap_dma(in_ap, for_custom_bir_dma=True)
    _idxs_ap = gp.lower_ap(idxs_ap)
    _out_ap = gp.lower_ap(out_ap)
    return gp.add_instruction(
        mybir.InstDMAGatherAnt(
            name=gp.bass.get_next_instruction_name(),
            ins=[*_in_ap, _idxs_ap, gp.lower_val_access(reg)],
            outs=[_out_ap], transpose=False, num_idxs=num_idxs,
            elem_size=elem_size, stride_bytes_256=stride_bytes_256,
            gen_mode=0, single_packet=False, queue_num=0,
            sbuf_tokens_per_rank=0, sbuf_free_dim_per_rank=0,
            sbuf_free_dim_pad_per_rank=0, sbuf_byte_offset=0,
        ))


# ------------------------------------------------------------- host layout

def _wrap16(rect):
    """[128, D] index rectangle -> [128, 8*D] int16 SBUF image.
    Gather ordinal i = c*128 + p reads wrapped[i % 16, i // 16]."""
    p, d = rect.shape
    assert p == P
    lin = rect.T.reshape(-1)                       # lin[c*128+p] = rect[p, c]
    w = lin.reshape(-1, 16).T                      # [16, 8*D]
    return np.tile(w, (8, 1)).astype(np.int16)


# ------------------------------------------------------------ device build

def _build_program(meta):
    N = meta["N"]; tiles = meta["tiles"]; ncores = meta["ncores"]
    npad = tiles * P
    split = meta["split"]
    t2rows = ncores * npad
    DA1, DB1, DA2, DB2 = meta["DA1"], meta["DB1"], meta["DA2"], meta["DB2"]
    C1 = 8 * (sum(DA1) + sum(DB1))
    C2 = 8 * (sum(DA2) + sum(DB2))
    HD = 64  # H1*D1
    EXT = 80
    ROW = 128

    nc = Bass()
    xT = nc.dram_tensor("xT", [P, N], BF16, kind="ExternalInput")
    w1e = nc.dram_tensor("w1e", [P, EXT], BF16, kind="ExternalInput")
    w2e = nc.dram_tensor("w2e", [HD, 18], BF16, kind="ExternalInput")
    b1i = nc.dram_tensor("b1i", [P, HD], F32, kind="ExternalInput")
    b2i = nc.dram_tensor("b2i", [P, 16], F32, kind="ExternalInput")
    idx1 = nc.dram_tensor("idx1", [P, max(C1, 8)], I16, kind="ExternalInput")
    idx2 = nc.dram_tensor("idx2", [P, max(C2, 8)], I16, kind="ExternalInput")
    idxo = nc.dram_tensor("idxo", [P, 16 * tiles], I16, kind="ExternalInput")
    m1 = nc.dram_tensor("m1", [P, tiles * 4], F32, kind="ExternalInput")
    mo = nc.dram_tensor("mo", [P, tiles * 2], F32, kind="ExternalInput")
    outd = nc.dram_tensor("out", [P, tiles * 16], F32, kind="ExternalOutput")

    T1 = nc.dram_tensor("T1", [N, ROW], BF16)
    T2sh = nc.dram_tensor("T2sh", [npad, ROW], BF16)
    T2 = nc.dram_tensor("T2", [t2rows, ROW], BF16)

    LR = mybir.ActivationFunctionType.Lrelu
    EXP = mybir.ActivationFunctionType.Exp
    LN = mybir.ActivationFunctionType.Ln
    ADD = mybir.AluOpType.add
    SUB = mybir.AluOpType.subtract
    MUL = mybir.AluOpType.mult

    with tile.TileContext(nc) as tc:
        with (
            tc.tile_pool(name="pers", bufs=1) as pp,
            tc.tile_pool(name="gat", bufs=2) as gp_,
            tc.tile_pool(name="wrk", bufs=2) as wp,
            tc.tile_pool(name="wrk1", bufs=1) as wp1,
            tc.tile_pool(name="psum", bufs=2, space="PSUM") as ps,
            tc.tile_pool(name="psum1", bufs=4, space="PSUM") as ps1,
        ):
            nc.gpsimd.load_library(library_config.mlp)
            # registers for gather sizes
            sizes = sorted({P * d for d in (DA1 + DB1 + DA2 + DB2 + [tiles]) if d > 0} | {P})
            regs = {s: nc.gpsimd.to_reg(s) for s in sizes}

            w1t = pp.tile([P, EXT], BF16)
            nc.sync.dma_start(w1t[:], w1e[:])
            w2t = pp.tile([HD, 18], BF16)
            nc.sync.dma_start(w2t[:], w2e[:])
            b1t = pp.tile([P, HD], F32)
            nc.sync.dma_start(b1t[:], b1i[:])
            b2t = pp.tile([P, 16], F32)
            nc.sync.dma_start(b2t[:], b2i[:])
            m1t = pp.tile([P, tiles * 4], F32)
            nc.sync.dma_start(m1t[:], m1[:])
            mot = pp.tile([P, tiles * 2], F32)
            nc.sync.dma_start(mot[:], mo[:])
            idxot = pp.tile([P, 16 * tiles], I16)
            nc.sync.dma_start(idxot[:], idxo[:])
            from concourse.masks import make_identity
            ident = pp.tile([P, P], BF16)
            make_identity(nc, ident[:])

            # ---------------- prepass: T1 rows [h|ls|ld] ----------------
            G512 = 512
            ngrp = (N + G512 - 1) // G512
            for g in range(ngrp):
                n0 = g * G512
                cols = min(G512, N - n0)
                xg = gp_.tile([P, G512], BF16, tag="xg")
                nc.sync.dma_start(xg[:, 0:cols], xT[:, n0:n0 + cols])
                nsub = (cols + P - 1) // P
                stg = gp_.tile([P, 4, EXT], BF16, tag="stg")
                for k in range(nsub):
                    c0 = k * P
                    cw = min(P, cols - c0)
                    mm = ps1.tile([P, EXT], F32, tag="ppp")
                    nc.tensor.matmul(mm[0:cw, :], lhsT=xg[:, c0:c0 + cw],
                                     rhs=w1t[:], start=True, stop=True)
                    nc.scalar.copy(stg[0:cw, k, :], mm[0:cw, :])
                nfull = cols // P
                if nfull > 0:
                    dst_ap = T1[n0:n0 + nfull * P, 0:EXT].rearrange(
                        '(k p) c -> p k c', p=P)
                    nc.sync.dma_start(dst_ap, stg[:, 0:nfull, :])
                rem = cols - nfull * P
                if rem > 0:
                    nc.sync.dma_start(T1[n0 + nfull * P:n0 + cols, 0:EXT],
                                      stg[0:rem, nfull, :])

            # -------------- own rows (self-loops, ld, ls_self) ----------
            zidx = pp.tile([P, 8], I16)
            nc.vector.memset(zidx[:], 0)
            rA0 = pp.tile([P, 1, EXT], BF16)
            _dma_gather(nc.gpsimd, rA0[:], T1[:, 0:EXT], zidx[:], P, regs[P], EXT)
            rB0 = pp.tile([P, 1, EXT], BF16)
            _dma_gather(nc.gpsimd, rB0[:], T1[split:, 0:EXT], zidx[:], P, regs[P], EXT)
            ownA = pp.tile([P, tiles, EXT], BF16)
            _dma_gather(nc.gpsimd, ownA[:], T1[:, 0:EXT],
                        idxot[:, 0:8 * tiles], P * tiles, regs[P * tiles], EXT)
            ownB = pp.tile([P, tiles, EXT], BF16)
            _dma_gather(nc.gpsimd, ownB[:], T1[split:, 0:EXT],
                        idxot[:, 8 * tiles:16 * tiles], P * tiles, regs[P * tiles], EXT)
            ownc = pp.tile([P, tiles, EXT], F32)
            nc.vector.tensor_tensor(out=ownc[:], in0=ownA[:], in1=ownB[:], op=ADD)
            tmpc = wp1.tile([P, tiles, EXT], F32, tag="tmpc")
            nc.vector.tensor_tensor(
                out=tmpc[:],
                in0=mot[:, 0:tiles].unsqueeze(2).broadcast_to([P, tiles, EXT]),
                in1=rA0[:, 0, :].unsqueeze(1).broadcast_to([P, tiles, EXT]),
                op=MUL)
            nc.vector.tensor_tensor(out=ownc[:], in0=ownc[:], in1=tmpc[:], op=SUB)
            tmpc2 = wp1.tile([P, tiles, EXT], F32, tag="tmpc")
            nc.vector.tensor_tensor(
                out=tmpc2[:],
                in0=mot[:, tiles:2 * tiles].unsqueeze(2).broadcast_to([P, tiles, EXT]),
                in1=rB0[:, 0, :].unsqueeze(1).broadcast_to([P, tiles, EXT]),
                op=MUL)
            nc.vector.tensor_tensor(out=ownc[:], in0=ownc[:], in1=tmpc2[:], op=SUB)
            # views into ownc
            # h_self = ownc[:, t, 0:64], ls_self = [64:72], ld = [72:80]

            # batched self-loop weights: wself_all [P, tiles, 8]
            eself = pp.tile([P, tiles, 8], F32)
            nc.vector.tensor_tensor(out=eself[:], in0=ownc[:, :, 64:72],
                                    in1=ownc[:, :, 72:80], op=ADD)
            nc.scalar.activation(eself[:], eself[:], LR, alpha=0.2)
            wself = pp.tile([P, tiles, 8], F32)
            nc.scalar.activation(wself[:], eself[:], EXP)
            msself = pp.tile([P, tiles, 64], F32)
            nc.vector.tensor_tensor(
                out=msself[:].rearrange('p t (h d) -> p t h d', h=8),
                in0=wself[:].unsqueeze(3).broadcast_to([P, tiles, 8, 8]),
                in1=ownc[:, :, 0:64].rearrange('p t (h d) -> p t h d', h=8),
                op=MUL)

            # batched pad corrections for layer 1
            def corr_batch(r0, mcols):
                e0 = wp1.tile([P, tiles, 8], F32, tag="e0")
                nc.vector.tensor_tensor(
                    out=e0[:],
                    in0=r0[:, 0, 64:72].unsqueeze(1).broadcast_to([P, tiles, 8]),
                    in1=ownc[:, :, 72:80], op=ADD)
                nc.scalar.activation(e0[:], e0[:], LR, alpha=0.2)
                nc.scalar.activation(e0[:], e0[:], EXP)
                mw = pp.tile([P, tiles, 8], F32, tag=f"mw{mcols}")
                nc.vector.tensor_tensor(
                    out=mw[:], in0=e0[:],
                    in1=m1t[:, mcols * tiles:(mcols + 1) * tiles]
                        .unsqueeze(2).broadcast_to([P, tiles, 8]),
                    op=MUL)
                mc = pp.tile([P, tiles, 64], F32, tag=f"mc{mcols}")
                nc.vector.tensor_tensor(
                    out=mc[:].rearrange('p t (h d) -> p t h d', h=8),
                    in0=mw[:].unsqueeze(3).broadcast_to([P, tiles, 8, 8]),
                    in1=r0[:, 0, 0:64].rearrange('p (h d) -> p h d', h=8).unsqueeze(1)
                        .broadcast_to([P, tiles, 8, 8]),
                    op=MUL)
                return mw, mc

            mwA, mcA = corr_batch(rA0, 0)
            mwB, mcB = corr_batch(rB0, 1)

            # ------------------------- layer 1 --------------------------
            T2stage = pp.tile([P, tiles, 18], BF16)
            h2f = pp.tile([P, tiles, 18], F32)
            offA = meta["offA1"]; offB = meta["offB1"]

            def structure_sums(gsl, D, ld_t, tag):
                """gather slots -> (wsum [P,8], msum [P,64]) partial."""
                e = wp.tile([P, D, 8], F32, tag=f"e{tag}")
                nc.vector.tensor_tensor(
                    out=e[:], in0=gsl[:, :, 64:72],
                    in1=ld_t.unsqueeze(1).broadcast_to([P, D, 8]),
                    op=ADD)
                nc.scalar.activation(e[:], e[:], LR, alpha=0.2)
                w = e
                nc.scalar.activation(w[:], e[:], EXP)
                ws = wp.tile([P, 8], F32, tag=f"ws{tag}")
                nc.vector.tensor_reduce(
                    out=ws[:], in_=w[:].rearrange('p s h -> p h s'),
                    axis=mybir.AxisListType.X, op=ADD)
                M = wp1.tile([P, D, 64], F32, tag="Mbig")
                nc.vector.tensor_tensor(
                    out=M[:].rearrange('p s (h d) -> p s h d', h=8),
                    in0=gsl[:, :, 0:64].rearrange('p s (h d) -> p s h d', h=8),
                    in1=w[:].unsqueeze(3).broadcast_to([P, D, 8, 8]),
                    op=MUL)
                cur = D
                while cur > 1:
                    half = cur // 2
                    nc.vector.tensor_tensor(
                        out=M[:, 0:half, :], in0=M[:, 0:half, :],
                        in1=M[:, half:2 * half, :], op=ADD)
                    if cur % 2:
                        nc.vector.tensor_tensor(
                            out=M[:, 0:1, :], in0=M[:, 0:1, :],
                            in1=M[:, cur - 1:cur, :], op=ADD)
                    cur = half
                return ws, M

            for t in range(tiles):
                ld_t = ownc[:, t, 72:80]
                DA, DB = DA1[t], DB1[t]
                den = wp.tile([P, 8], F32, tag="den")
                msum = wp.tile([P, 64], F32, tag="msum")
                nc.vector.tensor_copy(den[:], wself[:, t, :])
                nc.vector.tensor_copy(msum[:], msself[:, t, :])
                for (D, off, base_is_A) in ((DA, offA[t], True), (DB, offB[t], False)):
                    if D == 0:
                        continue
                    gt = gp_.tile([P, D, 72], BF16, tag=f"g{'A' if base_is_A else 'B'}")
                    src_t = T1[:, 0:72] if base_is_A else T1[split:, 0:72]
                    ist = gp_.tile([P, 8 * D], I16, tag=f"i{'A' if base_is_A else 'B'}")
                    nc.sync.dma_start(ist[:], idx1[:, off:off + 8 * D])
                    _dma_gather(nc.gpsimd, gt[:], src_t, ist[:], P * D, regs[P * D], 72)
                    ws, M = structure_sums(gt[:], D, ld_t, "A" if base_is_A else "B")
                    nc.vector.tensor_tensor(out=den[:], in0=den[:], in1=ws[:], op=ADD)
                    nc.vector.tensor_tensor(out=msum[:], in0=msum[:], in1=M[:, 0, :], op=ADD)
                # pad corrections
                nc.vector.tensor_tensor(out=den[:], in0=den[:], in1=mwA[:, t, :], op=SUB)
                nc.vector.tensor_tensor(out=den[:], in0=den[:], in1=mwB[:, t, :], op=SUB)
                nc.vector.tensor_tensor(out=msum[:], in0=msum[:], in1=mcA[:, t, :], op=SUB)
                nc.vector.tensor_tensor(out=msum[:], in0=msum[:], in1=mcB[:, t, :], op=SUB)
                # normalize + bias + elu
                nc.vector.tensor_scalar_add(den[:], den[:], 1e-16)
                rec = wp.tile([P, 8], F32, tag="rec")
                nc.vector.reciprocal(rec[:], den[:])
                z = wp.tile([P, 64], F32, tag="z")
                nc.vector.tensor_tensor(
                    out=z[:].rearrange('p (h d) -> p h d', h=8),
                    in0=msum[:].rearrange('p (h d) -> p h d', h=8),
                    in1=rec[:].unsqueeze(2).broadcast_to([P, 8, 8]),
                    op=MUL)
                nc.vector.tensor_tensor(out=z[:], in0=z[:], in1=b1t[:], op=ADD)
                zn = wp.tile([P, 64], F32, tag="zn")
                nc.vector.tensor_scalar_min(zn[:], z[:], 0.0)
                nc.scalar.activation(zn[:], zn[:], EXP)
                zp = wp.tile([P, 64], F32, tag="zp")
                nc.vector.tensor_scalar_max(zp[:], z[:], 0.0)
                nc.vector.tensor_tensor(out=zn[:], in0=zn[:], in1=zp[:], op=ADD)
                elu = wp.tile([P, 64], BF16, tag="elu")
                nc.vector.tensor_scalar_add(elu[:], zn[:], -1.0)
                # h2 = eluT @ w2e
                tps = ps.tile([64, P], BF16, tag="tp", space="PSUM")
                nc.tensor.transpose(tps[:], elu[:], ident[:])
                eluT = wp.tile([64, P], BF16, tag="eluT")
                nc.vector.tensor_copy(eluT[:], tps[:])
                h2p = ps.tile([P, 18], F32, tag="h2p", space="PSUM")
                nc.tensor.matmul(h2p[:], lhsT=eluT[:], rhs=w2t[:], start=True, stop=True)
                nc.scalar.copy(h2f[:, t, :], h2p[:])
                nc.vector.tensor_copy(T2stage[:, t, :], h2p[:])

            # stage T2 shard (processing order) and allgather
            nc.sync.dma_start(
                T2sh[:, 0:18].rearrange('(t p) c -> p t c', p=P), T2stage[:])
            nc.gpsimd.collective_compute(
                "AllGather", mybir.AluOpType.bypass,
                replica_groups=[list(range(meta["ncores"]))],
                ins=[T2sh[:]], outs=[T2[:]],
            )

            # ------------------------- layer 2 --------------------------
            r20 = pp.tile([P, 1, 17], BF16)
            _dma_gather(nc.gpsimd, r20[:], T2[:, 0:17], zidx[:], P, regs[P], 17)
            r21 = pp.tile([P, 1, 17], BF16)
            _dma_gather(nc.gpsimd, r21[:], T2[split:, 0:17], zidx[:], P, regs[P], 17)
            offA2 = meta["offA2"]; offB2 = meta["offB2"]
            out2 = pp.tile([P, tiles, 16], F32)

            for t in range(tiles):
                ld2_t = h2f[:, t, 17:18]
                den2 = wp.tile([P, 1], F32, tag="den2")
                ag2 = wp.tile([P, 16], F32, tag="ag2")
                # self loop
                e2s = wp.tile([P, 1], F32, tag="e2s")
                nc.vector.tensor_tensor(out=e2s[:], in0=h2f[:, t, 16:17],
                                        in1=ld2_t, op=ADD)
                nc.scalar.activation(e2s[:], e2s[:], LR, alpha=0.2)
                nc.scalar.activation(e2s[:], e2s[:], EXP)
                nc.vector.tensor_copy(den2[:], e2s[:])
                nc.vector.tensor_tensor(
                    out=ag2[:], in0=h2f[:, t, 0:16],
                    in1=e2s[:].broadcast_to([P, 16]), op=MUL)
                for (D, off, base_is_A) in ((DA2[t], offA2[t], True),
                                            (DB2[t], offB2[t], False)):
                    if D == 0:
                        continue
                    g2 = gp_.tile([P, D, 17], BF16, tag=f"g2{'A' if base_is_A else 'B'}")
                    src_t = T2[:, 0:17] if base_is_A else T2[split:, 0:17]
                    ist = gp_.tile([P, 8 * D], I16, tag=f"i2{'A' if base_is_A else 'B'}")
                    nc.sync.dma_start(ist[:], idx2[:, off:off + 8 * D])
                    _dma_gather(nc.gpsimd, g2[:], src_t, ist[:], P * D, regs[P * D], 17)
                    e2 = wp.tile([P, D], F32, tag=f"e2{'A' if base_is_A else 'B'}")
                    nc.vector.tensor_tensor(
                        out=e2[:], in0=g2[:, :, 16],
                        in1=ld2_t.broadcast_to([P, D]), op=ADD)
                    nc.scalar.activation(e2[:], e2[:], LR, alpha=0.2)
                    w2 = wp.tile([P, D], F32, tag=f"w2{'A' if base_is_A else 'B'}")
                    nc.scalar.activation(w2[:], e2[:], EXP)
                    ws2 = wp.tile([P, 1], F32, tag=f"ws2{'A' if base_is_A else 'B'}")
                    nc.vector.tensor_reduce(out=ws2[:], in_=w2[:],
                                            axis=mybir.AxisListType.X, op=ADD)
                    M2 = wp1.tile([P, D, 16], F32, tag="M2big")
                    nc.vector.tensor_tensor(
                        out=M2[:], in0=g2[:, :, 0:16],
                        in1=w2[:].unsqueeze(2).broadcast_to([P, D, 16]),
                        op=MUL)
                    cur = D
                    while cur > 1:
                        half = cur // 2
                        nc.vector.tensor_tensor(
                            out=M2[:, 0:half, :], in0=M2[:, 0:half, :],
                            in1=M2[:, half:2 * half, :], op=ADD)
                        if cur % 2:
                            nc.vector.tensor_tensor(
                                out=M2[:, 0:1, :], in0=M2[:, 0:1, :],
                                in1=M2[:, cur - 1:cur, :], op=ADD)
                        cur = half
                    nc.vector.tensor_tensor(out=den2[:], in0=den2[:], in1=ws2[:], op=ADD)
                    nc.vector.tensor_tensor(out=ag2[:], in0=ag2[:], in1=M2[:, 0, :], op=ADD)
                # pad corrections (layer 2)
                for (r2x, mcol) in ((r20, 2), (r21, 3)):
                    e20 = wp.tile([P, 1], F32, tag="e20")
                    nc.vector.tensor_tensor(out=e20[:], in0=r2x[:, 0, 16:17],
                                            in1=ld2_t, op=ADD)
                    nc.scalar.activation(e20[:], e20[:], LR, alpha=0.2)
                    nc.scalar.activation(e20[:], e20[:], EXP)
                    mw2 = wp.tile([P, 1], F32, tag="mw2")
                    nc.vector.tensor_tensor(
                        out=mw2[:], in0=e20[:],
                        in1=m1t[:, mcol * tiles + t:mcol * tiles + t + 1], op=MUL)
                    nc.vector.tensor_tensor(out=den2[:], in0=den2[:], in1=mw2[:], op=SUB)
                    mc2 = wp.tile([P, 16], F32, tag="mc2")
                    nc.vector.tensor_tensor(
                        out=mc2[:], in0=r2x[:, 0, 0:16],
                        in1=mw2[:].broadcast_to([P, 16]), op=MUL)
                    nc.vector.tensor_tensor(out=ag2[:], in0=ag2[:], in1=mc2[:], op=SUB)
                nc.vector.tensor_scalar_add(den2[:], den2[:], 1e-16)
                rec2 = wp.tile([P, 1], F32, tag="rec2")
                nc.vector.reciprocal(rec2[:], den2[:])
                nc.vector.tensor_tensor(out=ag2[:], in0=ag2[:],
                                        in1=rec2[:].broadcast_to([P, 16]), op=MUL)
                nc.vector.tensor_tensor(out=ag2[:], in0=ag2[:], in1=b2t[:], op=ADD)
                # log_softmax
                ex = wp.tile([P, 16], F32, tag="ex")
                nc.scalar.activation(ex[:], ag2[:], EXP)
                se = wp.tile([P, 1], F32, tag="se")
                nc.vector.tensor_reduce(out=se[:], in_=ex[:],
                                        axis=mybir.AxisListType.X, op=ADD)
                nc.scalar.activation(se[:], se[:], LN)
                nc.vector.tensor_tensor(
                    out=out2[:, t, :], in0=ag2[:],
                    in1=se[:].broadcast_to([P, 16]), op=SUB)

            nc.sync.dma_start(outd[:], out2[:].rearrange('p t c -> p (t c)'))

    _legalize_waits(nc)
    mybir.codegen_inst_isa_subclasses(nc)
    return nc


# ----------------------------------------------------------------- kernel

_CACHE = {}


def _prep(x, edge_index, W1, att_src1, att_dst1, b1, W2, att_src2, att_dst2, b2,
          ncores, tiles, split):
    N, F_IN = x.shape
    H1, D1 = att_src1.shape
    HD = H1 * D1
    C = W2.shape[1]
    npc = N // ncores
    npad = tiles * P

    src = np.asarray(edge_index[0], np.int64)
    dst = np.asarray(edge_index[1], np.int64)

    # derived params
    w1e = np.zeros((F_IN, 80), np.float32)
    w1e[:, 0:HD] = W1
    for h in range(H1):
        w1e[:, 64 + h] = W1[:, h * D1:(h + 1) * D1] @ att_src1[h]
        w1e[:, 72 + h] = W1[:, h * D1:(h + 1) * D1] @ att_dst1[h]
    w2e = np.zeros((HD, 18), np.float32)
    w2e[:, 0:C] = W2
    w2e[:, 16] = W2 @ att_src2[0]
    w2e[:, 17] = W2 @ att_dst2[0]

    owner = dst // npc
    dloc = dst - owner * npc
    deg = np.zeros((ncores, npc), np.int64)
    for c in range(ncores):
        deg[c] = np.bincount(dloc[owner == c], minlength=npc)
    # processing order: degree-desc, stable
    pos_to_local = [np.argsort(-deg[c], kind="stable") for c in range(ncores)]
    pos_of_local = []
    for c in range(ncores):
        q = np.empty(npc, np.int64)
        q[pos_to_local[c]] = np.arange(npc)
        pos_of_local.append(q)
    # global processing position of every node (for layer-2 table rows)
    pos2_of = np.empty(N, np.int64)
    for c in range(ncores):
        pos2_of[c * npc:(c + 1) * npc] = c * npad + pos_of_local[c]

    per_core = []
    for c in range(ncores):
        m = owner == c
        s_c, d_c = src[m], dloc[m]
        epos = pos_of_local[c][d_c]
        o = np.argsort(epos, kind="stable")
        s_s, e_s = s_c[o], epos[o]
        counts = np.bincount(e_s, minlength=npad)
        starts = np.concatenate([[0], np.cumsum(counts)])
        l2src = pos2_of[s_s]
        per_core.append(dict(src1=s_s, src2=l2src, starts=starts))

    def layer_meta(key):
        nA = np.zeros((ncores, npad), np.int64)
        nB = np.zeros((ncores, npad), np.int64)
        for c in range(ncores):
            pc = per_core[c]
            isA = pc[key] < split
            pc[f"isA_{key}"] = isA
            st = pc["starts"]
            epos_all = np.repeat(np.arange(npad), np.diff(st))
            nA[c] = np.bincount(epos_all[isA], minlength=npad)
            nB[c] = np.bincount(epos_all[~isA], minlength=npad)
        DA, DB = [], []
        for t in range(tiles):
            sl = slice(t * P, (t + 1) * P)
            da = int(nA[:, sl].max()); db = int(nB[:, sl].max())
            da = (da + 3) // 4 * 4 if da else 0
            db = (db + 3) // 4 * 4 if db else 0
            assert da <= 63 and db <= 63, (da, db)
            DA.append(da); DB.append(db)
        return nA, nB, DA, DB

    nA1, nB1, DA1, DB1 = layer_meta("src1")
    nA2, nB2, DA2, DB2 = layer_meta("src2")

    def build_idx(key, nA, nB, DA, DB):
        blocks, offA, offB = [], [], []
        colpos = 0
        for t in range(tiles):
            offA.append(colpos)
            colpos += 8 * DA[t]
            offB.append(colpos)
            colpos += 8 * DB[t]
        per_core_img = []
        for c in range(ncores):
            pc = per_core[c]
            segs = []
            st = pc["starts"]; s = pc[key]; isA = pc[f"isA_{key}"]
            for t in range(tiles):
                for (D, wantA, base) in ((DA[t], True, 0), (DB[t], False, split)):
                    if D == 0:
                        continue
                    rect = np.zeros((P, D), np.int64)
                    for p in range(P):
                        pos = t * P + p
                        vals = s[st[pos]:st[pos + 1]]
                        mk = isA[st[pos]:st[pos + 1]]
                        v = vals[mk] if wantA else vals[~mk]
                        rect[p, :len(v)] = v - base
                    segs.append(_wrap16(rect))
            img = np.concatenate(segs, axis=1) if segs else np.zeros((P, 8), np.int16)
            per_core_img.append(img)
        return per_core_img, offA, offB, colpos

    idx1_imgs, offA1, offB1, C1 = build_idx("src1", nA1, nB1, DA1, DB1)
    idx2_imgs, offA2, offB2, C2 = build_idx("src2", nA2, nB2, DA2, DB2)

    # pad-count tensors m1 = [mA1|mB1|mA2|mB2] as [P, tiles] each
    m1_imgs, mo_imgs, idxo_imgs = [], [], []
    for c in range(ncores):
        def padcnt(nX, DX):
            a = np.zeros((P, tiles), np.float32)
            for t in range(tiles):
                a[:, t] = DX[t] - nX[c, t * P:(t + 1) * P]
            return a
        m1_imgs.append(np.concatenate(
            [padcnt(nA1, DA1), padcnt(nB1, DB1),
             padcnt(nA2, DA2), padcnt(nB2, DB2)], axis=1))
        # own rows
        ids = c * npc + pos_to_local[c]              # [npc]
        ids_pad = np.zeros(npad, np.int64)
        ids_pad[:npc] = ids
        valid = np.zeros(npad, bool)
        valid[:npc] = True
        isA = (ids_pad < split) & valid
        isB = (ids_pad >= split) & valid
        oA = np.where(isA, ids_pad, 0)
        oB = np.where(isB, ids_pad - split, 0)
        moA = (~isA).astype(np.float32)
        moB = (~isB).astype(np.float32)
        rectA = oA.reshape(tiles, P).T.copy()        # [P, tiles]
        rectB = oB.reshape(tiles, P).T.copy()
        idxo_imgs.append(np.concatenate([_wrap16(rectA), _wrap16(rectB)], axis=1))
        mo_imgs.append(np.concatenate(
            [moA.reshape(tiles, P).T, moB.reshape(tiles, P).T], axis=1).astype(np.float32))

    meta = dict(N=N, tiles=tiles, ncores=ncores, split=split,
                DA1=DA1, DB1=DB1, DA2=DA2, DB2=DB2,
                offA1=offA1, offB1=offB1, offA2=offA2, offB2=offB2)

    xTb = np.ascontiguousarray(x.T).astype(ml_dtypes.bfloat16)
    in_maps = []
    for c in range(ncores):
        in_maps.append({
            "xT": xTb,
            "w1e": w1e.astype(ml_dtypes.bfloat16),
            "w2e": w2e.astype(ml_dtypes.bfloat16),
            "b1i": np.tile(np.asarray(b1, np.float32)[None, :], (P, 1)),
            "b2i": np.tile(np.asarray(b2, np.float32)[None, :], (P, 1)),
            "idx1": idx1_imgs[c] if C1 else np.zeros((P, 8), np.int16),
            "idx2": idx2_imgs[c] if C2 else np.zeros((P, 8), np.int16),
            "idxo": idxo_imgs[c],
            "m1": m1_imgs[c],
            "mo": mo_imgs[c],
        })
    perm = []  # output row for (core, position)
    for c in range(ncores):
        perm.append(c * npc + pos_to_local[c])
    return meta, in_maps, perm


def _run(meta, in_maps, perm, ncores, tiles, npc, C):
    key = "prog"
    if key not in _CACHE:
        _CACHE[key] = _build_program(meta)
    nc = _CACHE[key]
    t0 = time.time()
    res = run_bass_kernel_spmd(nc, in_maps, list(range(ncores)))
    t1 = time.time()
    _CACHE["last_wall"] = t1 - t0
    N = meta["N"]
    out = np.zeros((N, C), np.float32)
    for c in range(ncores):
        o = res.results[c]["out"].reshape(P, tiles, 16)   # [p, t, c]
        o = o.transpose(1, 0, 2).reshape(tiles * P, 16)   # position-major
        out[perm[c]] = o[:npc]
    return out


def kernel(x, edge_index, W1, att_src1, att_dst1, b1, W2, att_src2, att_dst2, b2):
    x = np.asarray(x, np.float32)
    meta, in_maps, perm = _prep(
        x, edge_index, np.asarray(W1, np.float32),
        np.asarray(att_src1, np.float32), np.asarray(att_dst1, np.float32),
        np.asarray(b1, np.float32), np.asarray(W2, np.float32),
        np.asarray(att_src2, np.float32), np.asarray(att_dst2, np.float32),
        np.asarray(b2, np.float32),
        ncores=8, tiles=49, split=32768)
    return _run(meta, in_maps, perm, 8, 49, x.shape[0] // 8, 16)


# revision 5
# speedup vs baseline: 8.0767x; 8.0767x over previous
"""Trainium2 Bass kernel for the 2-layer GAT (nn_GATNet).

Strategy (destination-sharded, 8 NeuronCores):
- Nodes (and their incoming edges) are partitioned across cores in
  contiguous id ranges.  Each core processes its destinations in
  degree-sorted tiles of 128.
- A prepass matmul builds T1 = [h(64) | ls(8) | ld(8)] rows for all
  nodes (replicated on every core).
- Per-edge source rows are fetched with the GPSIMD multi-index
  dma_gather (int16 indices -> the id space is split at 32768 into an
  A range (base 0) and B range (base 32768); rectangle padding points
  at row 0 of the range and is cancelled with host-provided pad
  counts).
- Segment softmax is computed densely per tile (dst on partitions,
  edge slots on the free dim); self-loops are applied densely from an
  own-row gather.
- Layer-2 table T2 = [h2(16) | ls2 | ld2] is built in processing
  order, staged to DRAM and AllGathered across the 8 cores; layer 2
  then repeats the gather/softmax with 17-column rows.
- log_softmax is computed per tile; the host undoes the processing
  permutation.
"""
import time

import numpy as np
import ml_dtypes

import concourse.tile as tile
import concourse.mybir as mybir
from concourse import library_config
from concourse.bass import Bass, exact_div
from concourse.bass_utils import run_bass_kernel_spmd

BF16 = mybir.dt.bfloat16
F32 = mybir.dt.float32
I16 = mybir.dt.int16

P = 128

# ---------------------------------------------------------------- wait fix
_ctr = [0]


def _legalize_waits(nc):
    """This walrus build rejects instructions with >1 semaphore wait.
    Split multi-wait instructions into single-wait NoOp carriers."""
    for fn in nc.m.functions:
        for bb in fn.blocks:
            insts = list(bb.instructions)
            out = []
            changed = False
            for inst in insts:
                si = inst.sync_info
                if si is not None and si.on_wait is not None and len(si.on_wait) > 1:
                    waits = list(si.on_wait)
                    ups = list(si.on_update) if si.on_update is not None else []
                    for w in waits[:-1]:
                        _ctr[0] += 1
                        nop = mybir.InstNoOp(name=f"waitnop-{_ctr[0]}")
                        nop.engine = inst.engine
                        nop.sync_info = mybir.SyncInfo(on_wait=[w], on_update=[])
                        out.append(nop)
                    inst.sync_info = mybir.SyncInfo(on_wait=[waits[-1]], on_update=ups)
                    changed = True
                out.append(inst)
            if changed:
                bb.instructions = out


def _dma_gather(gp, out_ap, in_ap, idxs_ap, num_idxs, reg, elem_size):
    """dma_gather without the (transpose-only) elem%256 assert.
    in_ap row stride must be a multiple of 256 bytes."""
    elem_step = in_ap.ap[0][0]
    stride_bytes_256 = exact_div(elem_step * mybir.dt.size(in_ap.dtype), 256)
    _in_ap = gp.lower_ap_dma(in_ap, for_custom_bir_dma=True)
    _idxs_ap = gp.lower_ap(idxs_ap)
    _out_ap = gp.lower_ap(out_ap)
    return gp.add_instruction(
        mybir.InstDMAGatherAnt(
            name=gp.bass.get_next_instruction_name(),
            ins=[*_in_ap, _idxs_ap, gp.lower_val_access(reg)],
            outs=[_out_ap], transpose=False, num_idxs=num_idxs,
            elem_size=elem_size, stride_bytes_256=stride_bytes_256,
            gen_mode=0, single_packet=False, queue_num=0,
            sbuf_tokens_per_rank=0, sbuf_free_dim_per_rank=0,
            sbuf_free_dim_pad_per_rank=0, sbuf_byte_offset=0,
        ))


# ------------------------------------------------------------- host layout

def _wrap16(rect):
    """[128, D] index rectangle -> [128, 8*D] int16 SBUF image.
    Gather ordinal i = c*128 + p reads wrapped[i % 16, i // 16]."""
    p, d = rect.shape
    assert p == P
    lin = rect.T.reshape(-1)                       # lin[c*128+p] = rect[p, c]
    w = lin.reshape(-1, 16).T                      # [16, 8*D]
    return np.tile(w, (8, 1)).astype(np.int16)


# ------------------------------------------------------------ device build

def _build_program(meta):
    N = meta["N"]; tiles = meta["tiles"]; ncores = meta["ncores"]
    npad = tiles * P
    split = meta["split"]
    t2rows = ncores * npad
    DA1, DB1, DA2, DB2 = meta["DA1"], meta["DB1"], meta["DA2"], meta["DB2"]
    C1 = 8 * (sum(DA1) + sum(DB1))
    C2 = 8 * (sum(DA2) + sum(DB2))
    HD = 64  # H1*D1
    EXT = 80
    ROW = 128

    nc = Bass()
    xT = nc.dram_tensor("xT", [P, N], BF16, kind="ExternalInput")
    w1e = nc.dram_tensor("w1e", [P, EXT], BF16, kind="ExternalInput")
    w2e = nc.dram_tensor("w2e", [HD, 18], BF16, kind="ExternalInput")
    b1i = nc.dram_tensor("b1i", [P, HD], F32, kind="ExternalInput")
    b2i = nc.dram_tensor("b2i", [P, 16], F32, kind="ExternalInput")
    idx1 = nc.dram_tensor("idx1", [P, max(C1, 8)], I16, kind="ExternalInput")
    idx2 = nc.dram_tensor("idx2", [P, max(C2, 8)], I16, kind="ExternalInput")
    idxo = nc.dram_tensor("idxo", [P, 16 * tiles], I16, kind="ExternalInput")
    m1 = nc.dram_tensor("m1", [P, tiles * 4], F32, kind="ExternalInput")
    mo = nc.dram_tensor("mo", [P, tiles * 2], F32, kind="ExternalInput")
    outd = nc.dram_tensor("out", [P, tiles * 16], F32, kind="ExternalOutput")

    T1 = nc.dram_tensor("T1", [N, ROW], BF16)
    T2sh = nc.dram_tensor("T2sh", [npad, ROW], BF16)
    T2 = nc.dram_tensor("T2", [t2rows, ROW], BF16, addr_space="Shared")

    LR = mybir.ActivationFunctionType.Lrelu
    EXP = mybir.ActivationFunctionType.Exp
    LN = mybir.ActivationFunctionType.Ln
    ADD = mybir.AluOpType.add
    SUB = mybir.AluOpType.subtract
    MUL = mybir.AluOpType.mult

    with tile.TileContext(nc) as tc:
        with (
            tc.tile_pool(name="pers", bufs=1) as pp,
            tc.tile_pool(name="gat", bufs=2) as gp_,
            tc.tile_pool(name="wrk", bufs=2) as wp,
            tc.tile_pool(name="wrk1", bufs=1) as wp1,
            tc.tile_pool(name="psum", bufs=2, space="PSUM") as ps,
            tc.tile_pool(name="psum1", bufs=4, space="PSUM") as ps1,
        ):
            nc.gpsimd.load_library(library_config.mlp)
            # registers for gather sizes
            sizes = sorted({P * d for d in (DA1 + DB1 + DA2 + DB2 + [tiles]) if d > 0} | {P})
            regs = {s: nc.gpsimd.to_reg(s) for s in sizes}

            w1t = pp.tile([P, EXT], BF16)
            nc.sync.dma_start(w1t[:], w1e[:])
            w2t = pp.tile([HD, 18], BF16)
            nc.sync.dma_start(w2t[:], w2e[:])
            b1t = pp.tile([P, HD], F32)
            nc.sync.dma_start(b1t[:], b1i[:])
            b2t = pp.tile([P, 16], F32)
            nc.sync.dma_start(b2t[:], b2i[:])
            m1t = pp.tile([P, tiles * 4], F32)
            nc.sync.dma_start(m1t[:], m1[:])
            mot = pp.tile([P, tiles * 2], F32)
            nc.sync.dma_start(mot[:], mo[:])
            idxot = pp.tile([P, 16 * tiles], I16)
            nc.sync.dma_start(idxot[:], idxo[:])
            from concourse.masks import make_identity
            ident = pp.tile([P, P], BF16)
            make_identity(nc, ident[:])

            # ---------------- prepass: T1 rows [h|ls|ld] ----------------
            G512 = 512
            ngrp = (N + G512 - 1) // G512
            for g in range(ngrp):
                n0 = g * G512
                cols = min(G512, N - n0)
                xg = gp_.tile([P, G512], BF16, tag="xg")
                nc.sync.dma_start(xg[:, 0:cols], xT[:, n0:n0 + cols])
                nsub = (cols + P - 1) // P
                stg = gp_.tile([P, 4, EXT], BF16, tag="stg")
                for k in range(nsub):
                    c0 = k * P
                    cw = min(P, cols - c0)
                    mm = ps1.tile([P, EXT], F32, tag="ppp")
                    nc.tensor.matmul(mm[0:cw, :], lhsT=xg[:, c0:c0 + cw],
                                     rhs=w1t[:], start=True, stop=True)
                    nc.scalar.copy(stg[0:cw, k, :], mm[0:cw, :])
                nfull = cols // P
                if nfull > 0:
                    dst_ap = T1[n0:n0 + nfull * P, 0:EXT].rearrange(
                        '(k p) c -> p k c', p=P)
                    nc.sync.dma_start(dst_ap, stg[:, 0:nfull, :])
                rem = cols - nfull * P
                if rem > 0:
                    nc.sync.dma_start(T1[n0 + nfull * P:n0 + cols, 0:EXT],
                                      stg[0:rem, nfull, :])

            # -------------- own rows (self-loops, ld, ls_self) ----------
            zidx = pp.tile([P, 8], I16)
            nc.vector.memset(zidx[:], 0)
            rA0 = pp.tile([P, 1, EXT], BF16)
            _dma_gather(nc.gpsimd, rA0[:], T1[:, 0:EXT], zidx[:], P, regs[P], EXT)
            rB0 = pp.tile([P, 1, EXT], BF16)
            _dma_gather(nc.gpsimd, rB0[:], T1[split:, 0:EXT], zidx[:], P, regs[P], EXT)
            ownA = pp.tile([P, tiles, EXT], BF16)
            _dma_gather(nc.gpsimd, ownA[:], T1[:, 0:EXT],
                        idxot[:, 0:8 * tiles], P * tiles, regs[P * tiles], EXT)
            ownB = pp.tile([P, tiles, EXT], BF16)
            _dma_gather(nc.gpsimd, ownB[:], T1[split:, 0:EXT],
                        idxot[:, 8 * tiles:16 * tiles], P * tiles, regs[P * tiles], EXT)
            ownc = pp.tile([P, tiles, EXT], F32)
            nc.vector.tensor_tensor(out=ownc[:], in0=ownA[:], in1=ownB[:], op=ADD)
            tmpc = wp1.tile([P, tiles, EXT], F32, tag="tmpc")
            nc.vector.tensor_tensor(
                out=tmpc[:],
                in0=mot[:, 0:tiles].unsqueeze(2).broadcast_to([P, tiles, EXT]),
                in1=rA0[:, 0, :].unsqueeze(1).broadcast_to([P, tiles, EXT]),
                op=MUL)
            nc.vector.tensor_tensor(out=ownc[:], in0=ownc[:], in1=tmpc[:], op=SUB)
            tmpc2 = wp1.tile([P, tiles, EXT], F32, tag="tmpc")
            nc.vector.tensor_tensor(
                out=tmpc2[:],
                in0=mot[:, tiles:2 * tiles].unsqueeze(2).broadcast_to([P, tiles, EXT]),
                in1=rB0[:, 0, :].unsqueeze(1).broadcast_to([P, tiles, EXT]),
                op=MUL)
            nc.vector.tensor_tensor(out=ownc[:], in0=ownc[:], in1=tmpc2[:], op=SUB)
            # views into ownc
            # h_self = ownc[:, t, 0:64], ls_self = [64:72], ld = [72:80]

            # batched self-loop weights: wself_all [P, tiles, 8]
            eself = pp.tile([P, tiles, 8], F32)
            nc.vector.tensor_tensor(out=eself[:], in0=ownc[:, :, 64:72],
                                    in1=ownc[:, :, 72:80], op=ADD)
            nc.scalar.activation(eself[:], eself[:], LR, alpha=0.2)
            wself = pp.tile([P, tiles, 8], F32)
            nc.scalar.activation(wself[:], eself[:], EXP)
            msself = pp.tile([P, tiles, 64], F32)
            nc.vector.tensor_tensor(
                out=msself[:].rearrange('p t (h d) -> p t h d', h=8),
                in0=wself[:].unsqueeze(3).broadcast_to([P, tiles, 8, 8]),
                in1=ownc[:, :, 0:64].rearrange('p t (h d) -> p t h d', h=8),
                op=MUL)

            # batched pad corrections for layer 1
            def corr_batch(r0, mcols):
                e0 = wp1.tile([P, tiles, 8], F32, tag="e0")
                nc.vector.tensor_tensor(
                    out=e0[:],
                    in0=r0[:, 0, 64:72].unsqueeze(1).broadcast_to([P, tiles, 8]),
                    in1=ownc[:, :, 72:80], op=ADD)
                nc.scalar.activation(e0[:], e0[:], LR, alpha=0.2)
                nc.scalar.activation(e0[:], e0[:], EXP)
                mw = pp.tile([P, tiles, 8], F32, tag=f"mw{mcols}")
                nc.vector.tensor_tensor(
                    out=mw[:], in0=e0[:],
                    in1=m1t[:, mcols * tiles:(mcols + 1) * tiles]
                        .unsqueeze(2).broadcast_to([P, tiles, 8]),
                    op=MUL)
                mc = pp.tile([P, tiles, 64], F32, tag=f"mc{mcols}")
                nc.vector.tensor_tensor(
                    out=mc[:].rearrange('p t (h d) -> p t h d', h=8),
                    in0=mw[:].unsqueeze(3).broadcast_to([P, tiles, 8, 8]),
                    in1=r0[:, 0, 0:64].rearrange('p (h d) -> p h d', h=8).unsqueeze(1)
                        .broadcast_to([P, tiles, 8, 8]),
                    op=MUL)
                return mw, mc

            mwA, mcA = corr_batch(rA0, 0)
            mwB, mcB = corr_batch(rB0, 1)

            # ------------------------- layer 1 --------------------------
            T2stage = pp.tile([P, tiles, 18], BF16)
            h2f = pp.tile([P, tiles, 18], F32)
            offA = meta["offA1"]; offB = meta["offB1"]

            def structure_sums(gsl, D, ld_t, tag):
                """gather slots -> (wsum [P,8], msum [P,64]) partial."""
                e = wp.tile([P, D, 8], F32, tag=f"e{tag}")
                nc.vector.tensor_tensor(
                    out=e[:], in0=gsl[:, :, 64:72],
                    in1=ld_t.unsqueeze(1).broadcast_to([P, D, 8]),
                    op=ADD)
                nc.scalar.activation(e[:], e[:], LR, alpha=0.2)
                w = e
                nc.scalar.activation(w[:], e[:], EXP)
                ws = wp.tile([P, 8], F32, tag=f"ws{tag}")
                nc.vector.tensor_reduce(
                    out=ws[:], in_=w[:].rearrange('p s h -> p h s'),
                    axis=mybir.AxisListType.X, op=ADD)
                M = wp1.tile([P, D, 64], F32, tag="Mbig")
                nc.vector.tensor_tensor(
                    out=M[:].rearrange('p s (h d) -> p s h d', h=8),
                    in0=gsl[:, :, 0:64].rearrange('p s (h d) -> p s h d', h=8),
                    in1=w[:].unsqueeze(3).broadcast_to([P, D, 8, 8]),
                    op=MUL)
                cur = D
                while cur > 1:
                    half = cur // 2
                    nc.vector.tensor_tensor(
                        out=M[:, 0:half, :], in0=M[:, 0:half, :],
                        in1=M[:, half:2 * half, :], op=ADD)
                    if cur % 2:
                        nc.vector.tensor_tensor(
                            out=M[:, 0:1, :], in0=M[:, 0:1, :],
                            in1=M[:, cur - 1:cur, :], op=ADD)
                    cur = half
                return ws, M

            for t in range(tiles):
                ld_t = ownc[:, t, 72:80]
                DA, DB = DA1[t], DB1[t]
                den = wp.tile([P, 8], F32, tag="den")
                msum = wp.tile([P, 64], F32, tag="msum")
                nc.vector.tensor_copy(den[:], wself[:, t, :])
                nc.vector.tensor_copy(msum[:], msself[:, t, :])
                for (D, off, base_is_A) in ((DA, offA[t], True), (DB, offB[t], False)):
                    if D == 0:
                        continue
                    gt = gp_.tile([P, D, 72], BF16, tag=f"g{'A' if base_is_A else 'B'}")
                    src_t = T1[:, 0:72] if base_is_A else T1[split:, 0:72]
                    ist = gp_.tile([P, 8 * D], I16, tag=f"i{'A' if base_is_A else 'B'}")
                    nc.sync.dma_start(ist[:], idx1[:, off:off + 8 * D])
                    _dma_gather(nc.gpsimd, gt[:], src_t, ist[:], P * D, regs[P * D], 72)
                    ws, M = structure_sums(gt[:], D, ld_t, "A" if base_is_A else "B")
                    nc.vector.tensor_tensor(out=den[:], in0=den[:], in1=ws[:], op=ADD)
                    nc.vector.tensor_tensor(out=msum[:], in0=msum[:], in1=M[:, 0, :], op=ADD)
                # pad corrections
                nc.vector.tensor_tensor(out=den[:], in0=den[:], in1=mwA[:, t, :], op=SUB)
                nc.vector.tensor_tensor(out=den[:], in0=den[:], in1=mwB[:, t, :], op=SUB)
                nc.vector.tensor_tensor(out=msum[:], in0=msum[:], in1=mcA[:, t, :], op=SUB)
                nc.vector.tensor_tensor(out=msum[:], in0=msum[:], in1=mcB[:, t, :], op=SUB)
                # normalize + bias + elu
                nc.vector.tensor_scalar_add(den[:], den[:], 1e-16)
                rec = wp.tile([P, 8], F32, tag="rec")
                nc.vector.reciprocal(rec[:], den[:])
                z = wp.tile([P, 64], F32, tag="z")
                nc.vector.tensor_tensor(
                    out=z[:].rearrange('p (h d) -> p h d', h=8),
                    in0=msum[:].rearrange('p (h d) -> p h d', h=8),
                    in1=rec[:].unsqueeze(2).broadcast_to([P, 8, 8]),
                    op=MUL)
                nc.vector.tensor_tensor(out=z[:], in0=z[:], in1=b1t[:], op=ADD)
                zn = wp.tile([P, 64], F32, tag="zn")
                nc.vector.tensor_scalar_min(zn[:], z[:], 0.0)
                nc.scalar.activation(zn[:], zn[:], EXP)
                zp = wp.tile([P, 64], F32, tag="zp")
                nc.vector.tensor_scalar_max(zp[:], z[:], 0.0)
                nc.vector.tensor_tensor(out=zn[:], in0=zn[:], in1=zp[:], op=ADD)
                elu = wp.tile([P, 64], BF16, tag="elu")
                nc.vector.tensor_scalar_add(elu[:], zn[:], -1.0)
                # h2 = eluT @ w2e
                tps = ps.tile([64, P], BF16, tag="tp", space="PSUM")
                nc.tensor.transpose(tps[:], elu[:], ident[:])
                eluT = wp.tile([64, P], BF16, tag="eluT")
                nc.vector.tensor_copy(eluT[:], tps[:])
                h2p = ps.tile([P, 18], F32, tag="h2p", space="PSUM")
                nc.tensor.matmul(h2p[:], lhsT=eluT[:], rhs=w2t[:], start=True, stop=True)
                nc.scalar.copy(h2f[:, t, :], h2p[:])
                nc.vector.tensor_copy(T2stage[:, t, :], h2p[:])

            # stage T2 shard (processing order) and allgather
            nc.sync.dma_start(
                T2sh[:, 0:18].rearrange('(t p) c -> p t c', p=P), T2stage[:])
            nc.gpsimd.collective_compute(
                "AllGather", mybir.AluOpType.bypass,
                replica_groups=[list(range(meta["ncores"]))],
                ins=[T2sh[:]], outs=[T2[:]],
            )

            # ------------------------- layer 2 --------------------------
            r20 = pp.tile([P, 1, 17], BF16)
            _dma_gather(nc.gpsimd, r20[:], T2[:, 0:17], zidx[:], P, regs[P], 17)
            r21 = pp.tile([P, 1, 17], BF16)
            _dma_gather(nc.gpsimd, r21[:], T2[split:, 0:17], zidx[:], P, regs[P], 17)
            offA2 = meta["offA2"]; offB2 = meta["offB2"]
            out2 = pp.tile([P, tiles, 16], F32)

            for t in range(tiles):
                ld2_t = h2f[:, t, 17:18]
                den2 = wp.tile([P, 1], F32, tag="den2")
                ag2 = wp.tile([P, 16], F32, tag="ag2")
                # self loop
                e2s = wp.tile([P, 1], F32, tag="e2s")
                nc.vector.tensor_tensor(out=e2s[:], in0=h2f[:, t, 16:17],
                                        in1=ld2_t, op=ADD)
                nc.scalar.activation(e2s[:], e2s[:], LR, alpha=0.2)
                nc.scalar.activation(e2s[:], e2s[:], EXP)
                nc.vector.tensor_copy(den2[:], e2s[:])
                nc.vector.tensor_tensor(
                    out=ag2[:], in0=h2f[:, t, 0:16],
                    in1=e2s[:].broadcast_to([P, 16]), op=MUL)
                for (D, off, base_is_A) in ((DA2[t], offA2[t], True),
                                            (DB2[t], offB2[t], False)):
                    if D == 0:
                        continue
                    g2 = gp_.tile([P, D, 17], BF16, tag=f"g2{'A' if base_is_A else 'B'}")
                    src_t = T2[:, 0:17] if base_is_A else T2[split:, 0:17]
                    ist = gp_.tile([P, 8 * D], I16, tag=f"i2{'A' if base_is_A else 'B'}")
                    nc.sync.dma_start(ist[:], idx2[:, off:off + 8 * D])
                    _dma_gather(nc.gpsimd, g2[:], src_t, ist[:], P * D, regs[P * D], 17)
                    e2 = wp.tile([P, D], F32, tag=f"e2{'A' if base_is_A else 'B'}")
                    nc.vector.tensor_tensor(
                        out=e2[:], in0=g2[:, :, 16],
                        in1=ld2_t.broadcast_to([P, D]), op=ADD)
                    nc.scalar.activation(e2[:], e2[:], LR, alpha=0.2)
                    w2 = wp.tile([P, D], F32, tag=f"w2{'A' if base_is_A else 'B'}")
                    nc.scalar.activation(w2[:], e2[:], EXP)
                    ws2 = wp.tile([P, 1], F32, tag=f"ws2{'A' if base_is_A else 'B'}")
                    nc.vector.tensor_reduce(out=ws2[:], in_=w2[:],
                                            axis=mybir.AxisListType.X, op=ADD)
                    M2 = wp1.tile([P, D, 16], F32, tag="M2big")
                    nc.vector.tensor_tensor(
                        out=M2[:], in0=g2[:, :, 0:16],
                        in1=w2[:].unsqueeze(2).broadcast_to([P, D, 16]),
                        op=MUL)
                    cur = D
                    while cur > 1:
                        half = cur // 2
                        nc.vector.tensor_tensor(
                            out=M2[:, 0:half, :], in0=M2[:, 0:half, :],
                            in1=M2[:, half:2 * half, :], op=ADD)
                        if cur % 2:
                            nc.vector.tensor_tensor(
                                out=M2[:, 0:1, :], in0=M2[:, 0:1, :],
                                in1=M2[:, cur - 1:cur, :], op=ADD)
                        cur = half
                    nc.vector.tensor_tensor(out=den2[:], in0=den2[:], in1=ws2[:], op=ADD)
                    nc.vector.tensor_tensor(out=ag2[:], in0=ag2[:], in1=M2[:, 0, :], op=ADD)
                # pad corrections (layer 2)
                for (r2x, mcol) in ((r20, 2), (r21, 3)):
                    e20 = wp.tile([P, 1], F32, tag="e20")
                    nc.vector.tensor_tensor(out=e20[:], in0=r2x[:, 0, 16:17],
                                            in1=ld2_t, op=ADD)
                    nc.scalar.activation(e20[:], e20[:], LR, alpha=0.2)
                    nc.scalar.activation(e20[:], e20[:], EXP)
                    mw2 = wp.tile([P, 1], F32, tag="mw2")
                    nc.vector.tensor_tensor(
                        out=mw2[:], in0=e20[:],
                        in1=m1t[:, mcol * tiles + t:mcol * tiles + t + 1], op=MUL)
                    nc.vector.tensor_tensor(out=den2[:], in0=den2[:], in1=mw2[:], op=SUB)
                    mc2 = wp.tile([P, 16], F32, tag="mc2")
                    nc.vector.tensor_tensor(
                        out=mc2[:], in0=r2x[:, 0, 0:16],
                        in1=mw2[:].broadcast_to([P, 16]), op=MUL)
                    nc.vector.tensor_tensor(out=ag2[:], in0=ag2[:], in1=mc2[:], op=SUB)
                nc.vector.tensor_scalar_add(den2[:], den2[:], 1e-16)
                rec2 = wp.tile([P, 1], F32, tag="rec2")
                nc.vector.reciprocal(rec2[:], den2[:])
                nc.vector.tensor_tensor(out=ag2[:], in0=ag2[:],
                                        in1=rec2[:].broadcast_to([P, 16]), op=MUL)
                nc.vector.tensor_tensor(out=ag2[:], in0=ag2[:], in1=b2t[:], op=ADD)
                # log_softmax
                ex = wp.tile([P, 16], F32, tag="ex")
                nc.scalar.activation(ex[:], ag2[:], EXP)
                se = wp.tile([P, 1], F32, tag="se")
                nc.vector.tensor_reduce(out=se[:], in_=ex[:],
                                        axis=mybir.AxisListType.X, op=ADD)
                nc.scalar.activation(se[:], se[:], LN)
                nc.vector.tensor_tensor(
                    out=out2[:, t, :], in0=ag2[:],
                    in1=se[:].broadcast_to([P, 16]), op=SUB)

            nc.sync.dma_start(outd[:], out2[:].rearrange('p t c -> p (t c)'))

    _legalize_waits(nc)
    mybir.codegen_inst_isa_subclasses(nc)
    return nc


# ----------------------------------------------------------------- kernel

_CACHE = {}


def _prep(x, edge_index, W1, att_src1, att_dst1, b1, W2, att_src2, att_dst2, b2,
          ncores, tiles, split):
    N, F_IN = x.shape
    H1, D1 = att_src1.shape
    HD = H1 * D1
    C = W2.shape[1]
    npc = N // ncores
    npad = tiles * P

    src = np.asarray(edge_index[0], np.int64)
    dst = np.asarray(edge_index[1], np.int64)

    # derived params
    w1e = np.zeros((F_IN, 80), np.float32)
    w1e[:, 0:HD] = W1
    for h in range(H1):
        w1e[:, 64 + h] = W1[:, h * D1:(h + 1) * D1] @ att_src1[h]
        w1e[:, 72 + h] = W1[:, h * D1:(h + 1) * D1] @ att_dst1[h]
    w2e = np.zeros((HD, 18), np.float32)
    w2e[:, 0:C] = W2
    w2e[:, 16] = W2 @ att_src2[0]
    w2e[:, 17] = W2 @ att_dst2[0]

    owner = dst // npc
    dloc = dst - owner * npc
    deg = np.zeros((ncores, npc), np.int64)
    for c in range(ncores):
        deg[c] = np.bincount(dloc[owner == c], minlength=npc)
    # processing order: degree-desc, stable
    pos_to_local = [np.argsort(-deg[c], kind="stable") for c in range(ncores)]
    pos_of_local = []
    for c in range(ncores):
        q = np.empty(npc, np.int64)
        q[pos_to_local[c]] = np.arange(npc)
        pos_of_local.append(q)
    # global processing position of every node (for layer-2 table rows)
    pos2_of = np.empty(N, np.int64)
    for c in range(ncores):
        pos2_of[c * npc:(c + 1) * npc] = c * npad + pos_of_local[c]

    per_core = []
    for c in range(ncores):
        m = owner == c
        s_c, d_c = src[m], dloc[m]
        epos = pos_of_local[c][d_c]
        o = np.argsort(epos, kind="stable")
        s_s, e_s = s_c[o], epos[o]
        counts = np.bincount(e_s, minlength=npad)
        starts = np.concatenate([[0], np.cumsum(counts)])
        l2src = pos2_of[s_s]
        per_core.append(dict(src1=s_s, src2=l2src, starts=starts))

    def layer_meta(key):
        nA = np.zeros((ncores, npad), np.int64)
        nB = np.zeros((ncores, npad), np.int64)
        for c in range(ncores):
            pc = per_core[c]
            isA = pc[key] < split
            pc[f"isA_{key}"] = isA
            st = pc["starts"]
            epos_all = np.repeat(np.arange(npad), np.diff(st))
            nA[c] = np.bincount(epos_all[isA], minlength=npad)
            nB[c] = np.bincount(epos_all[~isA], minlength=npad)
        DA, DB = [], []
        for t in range(tiles):
            sl = slice(t * P, (t + 1) * P)
            da = int(nA[:, sl].max()); db = int(nB[:, sl].max())
            da = (da + 3) // 4 * 4 if da else 0
            db = (db + 3) // 4 * 4 if db else 0
            assert da <= 63 and db <= 63, (da, db)
            DA.append(da); DB.append(db)
        return nA, nB, DA, DB

    nA1, nB1, DA1, DB1 = layer_meta("src1")
    nA2, nB2, DA2, DB2 = layer_meta("src2")

    def build_idx(key, nA, nB, DA, DB):
        blocks, offA, offB = [], [], []
        colpos = 0
        for t in range(tiles):
            offA.append(colpos)
            colpos += 8 * DA[t]
            offB.append(colpos)
            colpos += 8 * DB[t]
        per_core_img = []
        for c in range(ncores):
            pc = per_core[c]
            segs = []
            st = pc["starts"]; s = pc[key]; isA = pc[f"isA_{key}"]
            for t in range(tiles):
                for (D, wantA, base) in ((DA[t], True, 0), (DB[t], False, split)):
                    if D == 0:
                        continue
                    rect = np.zeros((P, D), np.int64)
                    for p in range(P):
                        pos = t * P + p
                        vals = s[st[pos]:st[pos + 1]]
                        mk = isA[st[pos]:st[pos + 1]]
                        v = vals[mk] if wantA else vals[~mk]
                        rect[p, :len(v)] = v - base
                    segs.append(_wrap16(rect))
            img = np.concatenate(segs, axis=1) if segs else np.zeros((P, 8), np.int16)
            per_core_img.append(img)
        return per_core_img, offA, offB, colpos

    idx1_imgs, offA1, offB1, C1 = build_idx("src1", nA1, nB1, DA1, DB1)
    idx2_imgs, offA2, offB2, C2 = build_idx("src2", nA2, nB2, DA2, DB2)

    # pad-count tensors m1 = [mA1|mB1|mA2|mB2] as [P, tiles] each
    m1_imgs, mo_imgs, idxo_imgs = [], [], []
    for c in range(ncores):
        def padcnt(nX, DX):
            a = np.zeros((P, tiles), np.float32)
            for t in range(tiles):
                a[:, t] = DX[t] - nX[c, t * P:(t + 1) * P]
            return a
        m1_imgs.append(np.concatenate(
            [padcnt(nA1, DA1), padcnt(nB1, DB1),
             padcnt(nA2, DA2), padcnt(nB2, DB2)], axis=1))
        # own rows
        ids = c * npc + pos_to_local[c]              # [npc]
        ids_pad = np.zeros(npad, np.int64)
        ids_pad[:npc] = ids
        valid = np.zeros(npad, bool)
        valid[:npc] = True
        isA = (ids_pad < split) & valid
        isB = (ids_pad >= split) & valid
        oA = np.where(isA, ids_pad, 0)
        oB = np.where(isB, ids_pad - split, 0)
        moA = (~isA).astype(np.float32)
        moB = (~isB).astype(np.float32)
        rectA = oA.reshape(tiles, P).T.copy()        # [P, tiles]
        rectB = oB.reshape(tiles, P).T.copy()
        idxo_imgs.append(np.concatenate([_wrap16(rectA), _wrap16(rectB)], axis=1))
        mo_imgs.append(np.concatenate(
            [moA.reshape(tiles, P).T, moB.reshape(tiles, P).T], axis=1).astype(np.float32))

    meta = dict(N=N, tiles=tiles, ncores=ncores, split=split,
                DA1=DA1, DB1=DB1, DA2=DA2, DB2=DB2,
                offA1=offA1, offB1=offB1, offA2=offA2, offB2=offB2)

    xTb = np.ascontiguousarray(x.T).astype(ml_dtypes.bfloat16)
    in_maps = []
    for c in range(ncores):
        in_maps.append({
            "xT": xTb,
            "w1e": w1e.astype(ml_dtypes.bfloat16),
            "w2e": w2e.astype(ml_dtypes.bfloat16),
            "b1i": np.tile(np.asarray(b1, np.float32)[None, :], (P, 1)),
            "b2i": np.tile(np.asarray(b2, np.float32)[None, :], (P, 1)),
            "idx1": idx1_imgs[c] if C1 else np.zeros((P, 8), np.int16),
            "idx2": idx2_imgs[c] if C2 else np.zeros((P, 8), np.int16),
            "idxo": idxo_imgs[c],
            "m1": m1_imgs[c],
            "mo": mo_imgs[c],
        })
    perm = []  # output row for (core, position)
    for c in range(ncores):
        perm.append(c * npc + pos_to_local[c])
    return meta, in_maps, perm


def _run(meta, in_maps, perm, ncores, tiles, npc, C):
    key = "prog"
    if key not in _CACHE:
        _CACHE[key] = _build_program(meta)
    nc = _CACHE[key]
    t0 = time.time()
    res = run_bass_kernel_spmd(nc, in_maps, list(range(ncores)))
    t1 = time.time()
    _CACHE["last_wall"] = t1 - t0
    N = meta["N"]
    out = np.zeros((N, C), np.float32)
    for c in range(ncores):
        o = res.results[c]["out"].reshape(P, tiles, 16)   # [p, t, c]
        o = o.transpose(1, 0, 2).reshape(tiles * P, 16)   # position-major
        out[perm[c]] = o[:npc]
    return out


def kernel(x, edge_index, W1, att_src1, att_dst1, b1, W2, att_src2, att_dst2, b2):
    x = np.asarray(x, np.float32)
    meta, in_maps, perm = _prep(
        x, edge_index, np.asarray(W1, np.float32),
        np.asarray(att_src1, np.float32), np.asarray(att_dst1, np.float32),
        np.asarray(b1, np.float32), np.asarray(W2, np.float32),
        np.asarray(att_src2, np.float32), np.asarray(att_dst2, np.float32),
        np.asarray(b2, np.float32),
        ncores=8, tiles=49, split=32768)
    return _run(meta, in_maps, perm, 8, 49, x.shape[0] // 8, 16)


# revision 8
# speedup vs baseline: 10.9985x; 1.3618x over previous
"""Trainium2 Bass kernel for the 2-layer GAT (nn_GATNet).

Strategy (destination-sharded, 8 NeuronCores):
- Nodes (and their incoming edges) are partitioned across cores in
  contiguous id ranges.  Each core processes its destinations in
  degree-sorted tiles of 128.
- A prepass matmul builds T1 = [h(64) | ls(8) | ld(8)] rows for all
  nodes (replicated on every core).
- Per-edge source rows are fetched with the GPSIMD multi-index
  dma_gather (int16 indices -> the id space is split at 32768 into an
  A range (base 0) and B range (base 32768); rectangle padding points
  at row 0 of the range and is cancelled with host-provided pad
  counts).
- Segment softmax is computed densely per tile (dst on partitions,
  edge slots on the free dim); self-loops are applied densely from an
  own-row gather.
- Layer-2 table T2 = [h2(16) | ls2 | ld2] is built in processing
  order, staged to DRAM and AllGathered across the 8 cores; layer 2
  then repeats the gather/softmax with 17-column rows.
- log_softmax is computed per tile; the host undoes the processing
  permutation.
"""
import time

import numpy as np
import ml_dtypes

import concourse.tile as tile
import concourse.mybir as mybir
from concourse import library_config
from concourse.bass import Bass, exact_div
from concourse.bass_utils import run_bass_kernel_spmd

BF16 = mybir.dt.bfloat16
F32 = mybir.dt.float32
I16 = mybir.dt.int16

P = 128

# ---------------------------------------------------------------- wait fix
_ctr = [0]


def _legalize_waits(nc):
    """This walrus build rejects instructions with >1 semaphore wait.
    Split multi-wait instructions into single-wait NoOp carriers."""
    for fn in nc.m.functions:
        for bb in fn.blocks:
            insts = list(bb.instructions)
            out = []
            changed = False
            for inst in insts:
                si = inst.sync_info
                if si is not None and si.on_wait is not None and len(si.on_wait) > 1:
                    waits = list(si.on_wait)
                    ups = list(si.on_update) if si.on_update is not None else []
                    for w in waits[:-1]:
                        _ctr[0] += 1
                        nop = mybir.InstNoOp(name=f"waitnop-{_ctr[0]}")
                        nop.engine = inst.engine
                        nop.sync_info = mybir.SyncInfo(on_wait=[w], on_update=[])
                        out.append(nop)
                    inst.sync_info = mybir.SyncInfo(on_wait=[waits[-1]], on_update=ups)
                    changed = True
                out.append(inst)
            if changed:
                bb.instructions = out


def _dma_gather(gp, out_ap, in_ap, idxs_ap, num_idxs, reg, elem_size):
    """dma_gather without the (transpose-only) elem%256 assert.
    in_ap row stride must be a multiple of 256 bytes."""
    elem_step = in_ap.ap[0][0]
    stride_bytes_256 = exact_div(elem_step * mybir.dt.size(in_ap.dtype), 256)
    _in_ap = gp.lower_ap_dma(in_ap, for_custom_bir_dma=True)
    _idxs_ap = gp.lower_ap(idxs_ap)
    _out_ap = gp.lower_ap(out_ap)
    return gp.add_instruction(
        mybir.InstDMAGatherAnt(
            name=gp.bass.get_next_instruction_name(),
            ins=[*_in_ap, _idxs_ap, gp.lower_val_access(reg)],
            outs=[_out_ap], transpose=False, num_idxs=num_idxs,
            elem_size=elem_size, stride_bytes_256=stride_bytes_256,
            gen_mode=0, single_packet=False, queue_num=0,
            sbuf_tokens_per_rank=0, sbuf_free_dim_per_rank=0,
            sbuf_free_dim_pad_per_rank=0, sbuf_byte_offset=0,
        ))


# ------------------------------------------------------------- host layout

def _wrap16(rect):
    """[128, D] index rectangle -> [128, 8*D] int16 SBUF image.
    Gather ordinal i = c*128 + p reads wrapped[i % 16, i // 16]."""
    p, d = rect.shape
    assert p == P
    lin = rect.T.reshape(-1)                       # lin[c*128+p] = rect[p, c]
    w = lin.reshape(-1, 16).T                      # [16, 8*D]
    return np.tile(w, (8, 1)).astype(np.int16)


# ------------------------------------------------------------ device build

def _build_program(meta):
    N = meta["N"]; tiles = meta["tiles"]; ncores = meta["ncores"]
    npad = tiles * P
    split = meta["split"]
    t2rows = ncores * npad
    DA1, DB1, DA2, DB2 = meta["DA1"], meta["DB1"], meta["DA2"], meta["DB2"]
    C1 = 8 * (sum(DA1) + sum(DB1))
    C2 = 8 * (sum(DA2) + sum(DB2))
    HD = 64  # H1*D1
    EXT = 80
    ROW = 128

    nc = Bass()
    xT = nc.dram_tensor("xT", [P, N], BF16, kind="ExternalInput")
    w1e = nc.dram_tensor("w1e", [P, EXT], BF16, kind="ExternalInput")
    w2e = nc.dram_tensor("w2e", [HD, 18], BF16, kind="ExternalInput")
    b1i = nc.dram_tensor("b1i", [P, HD], F32, kind="ExternalInput")
    b2i = nc.dram_tensor("b2i", [P, 16], F32, kind="ExternalInput")
    idx1 = nc.dram_tensor("idx1", [P, max(C1, 8)], I16, kind="ExternalInput")
    idx2 = nc.dram_tensor("idx2", [P, max(C2, 8)], I16, kind="ExternalInput")
    idxo = nc.dram_tensor("idxo", [P, 16 * tiles], I16, kind="ExternalInput")
    m1 = nc.dram_tensor("m1", [P, tiles * 4], F32, kind="ExternalInput")
    mo = nc.dram_tensor("mo", [P, tiles * 2], F32, kind="ExternalInput")
    outd = nc.dram_tensor("out", [P, tiles * 16], F32, kind="ExternalOutput")

    T1 = nc.dram_tensor("T1", [N, ROW], BF16)
    T2sh = nc.dram_tensor("T2sh", [npad, ROW], BF16)
    T2 = nc.dram_tensor("T2", [t2rows, ROW], BF16, addr_space="Shared")

    LR = mybir.ActivationFunctionType.Lrelu
    EXP = mybir.ActivationFunctionType.Exp
    LN = mybir.ActivationFunctionType.Ln
    ADD = mybir.AluOpType.add
    SUB = mybir.AluOpType.subtract
    MUL = mybir.AluOpType.mult

    with tile.TileContext(nc) as tc:
        with (
            tc.tile_pool(name="pers", bufs=1) as pp,
            tc.tile_pool(name="gat", bufs=2) as gp_,
            tc.tile_pool(name="wrk", bufs=2) as wp,
            tc.tile_pool(name="wrk1", bufs=1) as wp1,
            tc.tile_pool(name="psum", bufs=2, space="PSUM") as ps,
            tc.tile_pool(name="psum1", bufs=4, space="PSUM") as ps1,
        ):
            nc.gpsimd.load_library(library_config.mlp)
            # registers for gather sizes
            sizes = sorted({P * d for d in (DA1 + DB1 + DA2 + DB2 + [tiles]) if d > 0} | {P})
            regs = {s: nc.gpsimd.to_reg(s) for s in sizes}

            w1t = pp.tile([P, EXT], BF16)
            nc.sync.dma_start(w1t[:], w1e[:])
            w2t = pp.tile([HD, 18], BF16)
            nc.sync.dma_start(w2t[:], w2e[:])
            b1t = pp.tile([P, HD], F32)
            nc.sync.dma_start(b1t[:], b1i[:])
            b2t = pp.tile([P, 16], F32)
            nc.sync.dma_start(b2t[:], b2i[:])
            m1t = pp.tile([P, tiles * 4], F32)
            nc.sync.dma_start(m1t[:], m1[:])
            mot = pp.tile([P, tiles * 2], F32)
            nc.sync.dma_start(mot[:], mo[:])
            idxot = pp.tile([P, 16 * tiles], I16)
            nc.sync.dma_start(idxot[:], idxo[:])
            allidx = pp.tile([P, max(max(C1, C2), 8)], I16, tag="allidx")
            nc.sync.dma_start(allidx[:, 0:max(C1, 8)], idx1[:])
            from concourse.masks import make_identity
            ident = pp.tile([P, P], BF16)
            make_identity(nc, ident[:])

            # ---------------- prepass: T1 rows [h|ls|ld] ----------------
            G512 = 512
            ngrp = (N + G512 - 1) // G512
            for g in range(ngrp):
                n0 = g * G512
                cols = min(G512, N - n0)
                xg = gp_.tile([P, G512], BF16, tag="xg")
                nc.sync.dma_start(xg[:, 0:cols], xT[:, n0:n0 + cols])
                nsub = (cols + P - 1) // P
                stg = gp_.tile([P, 4, EXT], BF16, tag="stg")
                for k in range(nsub):
                    c0 = k * P
                    cw = min(P, cols - c0)
                    mm = ps1.tile([P, EXT], F32, tag="ppp")
                    nc.tensor.matmul(mm[0:cw, :], lhsT=xg[:, c0:c0 + cw],
                                     rhs=w1t[:], start=True, stop=True)
                    nc.scalar.copy(stg[0:cw, k, :], mm[0:cw, :])
                nfull = cols // P
                if nfull > 0:
                    dst_ap = T1[n0:n0 + nfull * P, 0:EXT].rearrange(
                        '(k p) c -> p k c', p=P)
                    nc.sync.dma_start(dst_ap, stg[:, 0:nfull, :])
                rem = cols - nfull * P
                if rem > 0:
                    nc.sync.dma_start(T1[n0 + nfull * P:n0 + cols, 0:EXT],
                                      stg[0:rem, nfull, :])

            # -------------- own rows (self-loops, ld, ls_self) ----------
            zidx = pp.tile([P, 8], I16)
            nc.vector.memset(zidx[:], 0)
            rA0 = pp.tile([P, 1, EXT], BF16)
            _dma_gather(nc.gpsimd, rA0[:], T1[:, 0:EXT], zidx[:], P, regs[P], EXT)
            rB0 = pp.tile([P, 1, EXT], BF16)
            _dma_gather(nc.gpsimd, rB0[:], T1[split:, 0:EXT], zidx[:], P, regs[P], EXT)
            ownA = pp.tile([P, tiles, EXT], BF16)
            _dma_gather(nc.gpsimd, ownA[:], T1[:, 0:EXT],
                        idxot[:, 0:8 * tiles], P * tiles, regs[P * tiles], EXT)
            ownB = pp.tile([P, tiles, EXT], BF16)
            _dma_gather(nc.gpsimd, ownB[:], T1[split:, 0:EXT],
                        idxot[:, 8 * tiles:16 * tiles], P * tiles, regs[P * tiles], EXT)
            ownc = pp.tile([P, tiles, EXT], F32)
            nc.vector.tensor_tensor(out=ownc[:], in0=ownA[:], in1=ownB[:], op=ADD)
            tmpc = wp1.tile([P, tiles, EXT], F32, tag="tmpc")
            nc.vector.tensor_tensor(
                out=tmpc[:],
                in0=mot[:, 0:tiles].unsqueeze(2).broadcast_to([P, tiles, EXT]),
                in1=rA0[:, 0, :].unsqueeze(1).broadcast_to([P, tiles, EXT]),
                op=MUL)
            nc.vector.tensor_tensor(out=ownc[:], in0=ownc[:], in1=tmpc[:], op=SUB)
            tmpc2 = wp1.tile([P, tiles, EXT], F32, tag="tmpc")
            nc.vector.tensor_tensor(
                out=tmpc2[:],
                in0=mot[:, tiles:2 * tiles].unsqueeze(2).broadcast_to([P, tiles, EXT]),
                in1=rB0[:, 0, :].unsqueeze(1).broadcast_to([P, tiles, EXT]),
                op=MUL)
            nc.vector.tensor_tensor(out=ownc[:], in0=ownc[:], in1=tmpc2[:], op=SUB)
            # views into ownc
            # h_self = ownc[:, t, 0:64], ls_self = [64:72], ld = [72:80]

            # batched self-loop weights: wself_all [P, tiles, 8]
            eself = pp.tile([P, tiles, 8], F32)
            nc.vector.tensor_tensor(out=eself[:], in0=ownc[:, :, 64:72],
                                    in1=ownc[:, :, 72:80], op=ADD)
            nc.scalar.activation(eself[:], eself[:], LR, alpha=0.2)
            wself = pp.tile([P, tiles, 8], F32)
            nc.scalar.activation(wself[:], eself[:], EXP)
            msself = pp.tile([P, tiles, 64], F32)
            nc.vector.tensor_tensor(
                out=msself[:].rearrange('p t (h d) -> p t h d', h=8),
                in0=wself[:].unsqueeze(3).broadcast_to([P, tiles, 8, 8]),
                in1=ownc[:, :, 0:64].rearrange('p t (h d) -> p t h d', h=8),
                op=MUL)

            # batched pad corrections for layer 1
            def corr_batch(r0, mcols):
                e0 = wp1.tile([P, tiles, 8], F32, tag="e0")
                nc.vector.tensor_tensor(
                    out=e0[:],
                    in0=r0[:, 0, 64:72].unsqueeze(1).broadcast_to([P, tiles, 8]),
                    in1=ownc[:, :, 72:80], op=ADD)
                nc.scalar.activation(e0[:], e0[:], LR, alpha=0.2)
                nc.scalar.activation(e0[:], e0[:], EXP)
                mw = pp.tile([P, tiles, 8], F32, tag=f"mw{mcols}")
                nc.vector.tensor_tensor(
                    out=mw[:], in0=e0[:],
                    in1=m1t[:, mcols * tiles:(mcols + 1) * tiles]
                        .unsqueeze(2).broadcast_to([P, tiles, 8]),
                    op=MUL)
                mc = pp.tile([P, tiles, 64], F32, tag=f"mc{mcols}")
                nc.vector.tensor_tensor(
                    out=mc[:].rearrange('p t (h d) -> p t h d', h=8),
                    in0=mw[:].unsqueeze(3).broadcast_to([P, tiles, 8, 8]),
                    in1=r0[:, 0, 0:64].rearrange('p (h d) -> p h d', h=8).unsqueeze(1)
                        .broadcast_to([P, tiles, 8, 8]),
                    op=MUL)
                return mw, mc

            mwA, mcA = corr_batch(rA0, 0)
            mwB, mcB = corr_batch(rB0, 1)

            # ------------------------- layer 1 --------------------------
            T2stage = pp.tile([P, tiles, 18], BF16)
            h2f = pp.tile([P, tiles, 18], F32)
            offA = meta["offA1"]; offB = meta["offB1"]

            def structure_sums(gsl, D, ld_t, tag):
                """gather slots -> (wsum [P,8], msum [P,64]) partial."""
                e = wp.tile([P, D, 8], F32, tag=f"e{tag}")
                nc.vector.tensor_tensor(
                    out=e[:], in0=gsl[:, :, 64:72],
                    in1=ld_t.unsqueeze(1).broadcast_to([P, D, 8]),
                    op=ADD)
                nc.scalar.activation(e[:], e[:], LR, alpha=0.2)
                w = e
                nc.scalar.activation(w[:], e[:], EXP)
                ws = wp.tile([P, 8], F32, tag=f"ws{tag}")
                nc.vector.tensor_reduce(
                    out=ws[:], in_=w[:].rearrange('p s h -> p h s'),
                    axis=mybir.AxisListType.X, op=ADD)
                M = wp1.tile([P, D, 64], F32, tag="Mbig")
                nc.vector.tensor_tensor(
                    out=M[:].rearrange('p s (h d) -> p s h d', h=8),
                    in0=gsl[:, :, 0:64].rearrange('p s (h d) -> p s h d', h=8),
                    in1=w[:].unsqueeze(3).broadcast_to([P, D, 8, 8]),
                    op=MUL)
                cur = D
                while cur > 1:
                    half = cur // 2
                    nc.vector.tensor_tensor(
                        out=M[:, 0:half, :], in0=M[:, 0:half, :],
                        in1=M[:, half:2 * half, :], op=ADD)
                    if cur % 2:
                        nc.vector.tensor_tensor(
                            out=M[:, 0:1, :], in0=M[:, 0:1, :],
                            in1=M[:, cur - 1:cur, :], op=ADD)
                    cur = half
                return ws, M

            for t in range(tiles):
                ld_t = ownc[:, t, 72:80]
                DA, DB = DA1[t], DB1[t]
                den = wp.tile([P, 8], F32, tag="den")
                msum = wp.tile([P, 64], F32, tag="msum")
                nc.vector.tensor_copy(den[:], wself[:, t, :])
                nc.vector.tensor_copy(msum[:], msself[:, t, :])
                for (D, off, base_is_A) in ((DA, offA[t], True), (DB, offB[t], False)):
                    if D == 0:
                        continue
                    gt = gp_.tile([P, D, 72], BF16, tag=f"g{'A' if base_is_A else 'B'}")
                    src_t = T1[:, 0:72] if base_is_A else T1[split:, 0:72]
                    _dma_gather(nc.gpsimd, gt[:], src_t,
                                allidx[:, off:off + 8 * D], P * D, regs[P * D], 72)
                    ws, M = structure_sums(gt[:], D, ld_t, "A" if base_is_A else "B")
                    nc.vector.tensor_tensor(out=den[:], in0=den[:], in1=ws[:], op=ADD)
                    nc.vector.tensor_tensor(out=msum[:], in0=msum[:], in1=M[:, 0, :], op=ADD)
                # pad corrections
                nc.vector.tensor_tensor(out=den[:], in0=den[:], in1=mwA[:, t, :], op=SUB)
                nc.vector.tensor_tensor(out=den[:], in0=den[:], in1=mwB[:, t, :], op=SUB)
                nc.vector.tensor_tensor(out=msum[:], in0=msum[:], in1=mcA[:, t, :], op=SUB)
                nc.vector.tensor_tensor(out=msum[:], in0=msum[:], in1=mcB[:, t, :], op=SUB)
                # normalize + bias + elu
                nc.vector.tensor_scalar_add(den[:], den[:], 1e-16)
                rec = wp.tile([P, 8], F32, tag="rec")
                nc.vector.reciprocal(rec[:], den[:])
                z = wp.tile([P, 64], F32, tag="z")
                nc.vector.tensor_tensor(
                    out=z[:].rearrange('p (h d) -> p h d', h=8),
                    in0=msum[:].rearrange('p (h d) -> p h d', h=8),
                    in1=rec[:].unsqueeze(2).broadcast_to([P, 8, 8]),
                    op=MUL)
                nc.vector.tensor_tensor(out=z[:], in0=z[:], in1=b1t[:], op=ADD)
                zn = wp.tile([P, 64], F32, tag="zn")
                nc.vector.tensor_scalar_min(zn[:], z[:], 0.0)
                nc.scalar.activation(zn[:], zn[:], EXP)
                zp = wp.tile([P, 64], F32, tag="zp")
                nc.vector.tensor_scalar_max(zp[:], z[:], 0.0)
                nc.vector.tensor_tensor(out=zn[:], in0=zn[:], in1=zp[:], op=ADD)
                elu = wp.tile([P, 64], BF16, tag="elu")
                nc.vector.tensor_scalar_add(elu[:], zn[:], -1.0)
                # h2 = eluT @ w2e
                tps = ps.tile([64, P], BF16, tag="tp", space="PSUM")
                nc.tensor.transpose(tps[:], elu[:], ident[:])
                eluT = wp.tile([64, P], BF16, tag="eluT")
                nc.vector.tensor_copy(eluT[:], tps[:])
                h2p = ps.tile([P, 18], F32, tag="h2p", space="PSUM")
                nc.tensor.matmul(h2p[:], lhsT=eluT[:], rhs=w2t[:], start=True, stop=True)
                nc.scalar.copy(h2f[:, t, :], h2p[:])
                nc.vector.tensor_copy(T2stage[:, t, :], h2p[:])

            # stage T2 shard (processing order) and allgather
            nc.sync.dma_start(
                T2sh[:, 0:18].rearrange('(t p) c -> p t c', p=P), T2stage[:])
            nc.gpsimd.collective_compute(
                "AllGather", mybir.AluOpType.bypass,
                replica_groups=[list(range(meta["ncores"]))],
                ins=[T2sh[:]], outs=[T2[:]],
            )

            # ------------------------- layer 2 --------------------------
            nc.sync.dma_start(allidx[:, 0:max(C2, 8)], idx2[:])
            r20 = pp.tile([P, 1, 17], BF16)
            _dma_gather(nc.gpsimd, r20[:], T2[:, 0:17], zidx[:], P, regs[P], 17)
            r21 = pp.tile([P, 1, 17], BF16)
            _dma_gather(nc.gpsimd, r21[:], T2[split:, 0:17], zidx[:], P, regs[P], 17)
            offA2 = meta["offA2"]; offB2 = meta["offB2"]
            out2 = pp.tile([P, tiles, 16], F32)

            for t in range(tiles):
                ld2_t = h2f[:, t, 17:18]
                den2 = wp.tile([P, 1], F32, tag="den2")
                ag2 = wp.tile([P, 16], F32, tag="ag2")
                # self loop
                e2s = wp.tile([P, 1], F32, tag="e2s")
                nc.vector.tensor_tensor(out=e2s[:], in0=h2f[:, t, 16:17],
                                        in1=ld2_t, op=ADD)
                nc.scalar.activation(e2s[:], e2s[:], LR, alpha=0.2)
                nc.scalar.activation(e2s[:], e2s[:], EXP)
                nc.vector.tensor_copy(den2[:], e2s[:])
                nc.vector.tensor_tensor(
                    out=ag2[:], in0=h2f[:, t, 0:16],
                    in1=e2s[:].broadcast_to([P, 16]), op=MUL)
                for (D, off, base_is_A) in ((DA2[t], offA2[t], True),
                                            (DB2[t], offB2[t], False)):
                    if D == 0:
                        continue
                    g2 = gp_.tile([P, D, 17], BF16, tag=f"g2{'A' if base_is_A else 'B'}")
                    src_t = T2[:, 0:17] if base_is_A else T2[split:, 0:17]
                    _dma_gather(nc.gpsimd, g2[:], src_t,
                                allidx[:, off:off + 8 * D], P * D, regs[P * D], 17)
                    e2 = wp.tile([P, D], F32, tag=f"e2{'A' if base_is_A else 'B'}")
                    nc.vector.tensor_tensor(
                        out=e2[:], in0=g2[:, :, 16],
                        in1=ld2_t.broadcast_to([P, D]), op=ADD)
                    nc.scalar.activation(e2[:], e2[:], LR, alpha=0.2)
                    w2 = wp.tile([P, D], F32, tag=f"w2{'A' if base_is_A else 'B'}")
                    nc.scalar.activation(w2[:], e2[:], EXP)
                    ws2 = wp.tile([P, 1], F32, tag=f"ws2{'A' if base_is_A else 'B'}")
                    nc.vector.tensor_reduce(out=ws2[:], in_=w2[:],
                                            axis=mybir.AxisListType.X, op=ADD)
                    M2 = wp1.tile([P, D, 16], F32, tag="M2big")
                    nc.vector.tensor_tensor(
                        out=M2[:], in0=g2[:, :, 0:16],
                        in1=w2[:].unsqueeze(2).broadcast_to([P, D, 16]),
                        op=MUL)
                    cur = D
                    while cur > 1:
                        half = cur // 2
                        nc.vector.tensor_tensor(
                            out=M2[:, 0:half, :], in0=M2[:, 0:half, :],
                            in1=M2[:, half:2 * half, :], op=ADD)
                        if cur % 2:
                            nc.vector.tensor_tensor(
                                out=M2[:, 0:1, :], in0=M2[:, 0:1, :],
                                in1=M2[:, cur - 1:cur, :], op=ADD)
                        cur = half
                    nc.vector.tensor_tensor(out=den2[:], in0=den2[:], in1=ws2[:], op=ADD)
                    nc.vector.tensor_tensor(out=ag2[:], in0=ag2[:], in1=M2[:, 0, :], op=ADD)
                # pad corrections (layer 2)
                for (r2x, mcol) in ((r20, 2), (r21, 3)):
                    e20 = wp.tile([P, 1], F32, tag="e20")
                    nc.vector.tensor_tensor(out=e20[:], in0=r2x[:, 0, 16:17],
                                            in1=ld2_t, op=ADD)
                    nc.scalar.activation(e20[:], e20[:], LR, alpha=0.2)
                    nc.scalar.activation(e20[:], e20[:], EXP)
                    mw2 = wp.tile([P, 1], F32, tag="mw2")
                    nc.vector.tensor_tensor(
                        out=mw2[:], in0=e20[:],
                        in1=m1t[:, mcol * tiles + t:mcol * tiles + t + 1], op=MUL)
                    nc.vector.tensor_tensor(out=den2[:], in0=den2[:], in1=mw2[:], op=SUB)
                    mc2 = wp.tile([P, 16], F32, tag="mc2")
                    nc.vector.tensor_tensor(
                        out=mc2[:], in0=r2x[:, 0, 0:16],
                        in1=mw2[:].broadcast_to([P, 16]), op=MUL)
                    nc.vector.tensor_tensor(out=ag2[:], in0=ag2[:], in1=mc2[:], op=SUB)
                nc.vector.tensor_scalar_add(den2[:], den2[:], 1e-16)
                rec2 = wp.tile([P, 1], F32, tag="rec2")
                nc.vector.reciprocal(rec2[:], den2[:])
                nc.vector.tensor_tensor(out=ag2[:], in0=ag2[:],
                                        in1=rec2[:].broadcast_to([P, 16]), op=MUL)
                nc.vector.tensor_tensor(out=ag2[:], in0=ag2[:], in1=b2t[:], op=ADD)
                # log_softmax
                ex = wp.tile([P, 16], F32, tag="ex")
                nc.scalar.activation(ex[:], ag2[:], EXP)
                se = wp.tile([P, 1], F32, tag="se")
                nc.vector.tensor_reduce(out=se[:], in_=ex[:],
                                        axis=mybir.AxisListType.X, op=ADD)
                nc.scalar.activation(se[:], se[:], LN)
                nc.vector.tensor_tensor(
                    out=out2[:, t, :], in0=ag2[:],
                    in1=se[:].broadcast_to([P, 16]), op=SUB)

            nc.sync.dma_start(outd[:], out2[:].rearrange('p t c -> p (t c)'))

    _legalize_waits(nc)
    mybir.codegen_inst_isa_subclasses(nc)
    return nc


# ----------------------------------------------------------------- kernel

_CACHE = {}


def _prep(x, edge_index, W1, att_src1, att_dst1, b1, W2, att_src2, att_dst2, b2,
          ncores, tiles, split):
    N, F_IN = x.shape
    H1, D1 = att_src1.shape
    HD = H1 * D1
    C = W2.shape[1]
    npc = N // ncores
    npad = tiles * P

    src = np.asarray(edge_index[0], np.int64)
    dst = np.asarray(edge_index[1], np.int64)

    # derived params
    w1e = np.zeros((F_IN, 80), np.float32)
    w1e[:, 0:HD] = W1
    for h in range(H1):
        w1e[:, 64 + h] = W1[:, h * D1:(h + 1) * D1] @ att_src1[h]
        w1e[:, 72 + h] = W1[:, h * D1:(h + 1) * D1] @ att_dst1[h]
    w2e = np.zeros((HD, 18), np.float32)
    w2e[:, 0:C] = W2
    w2e[:, 16] = W2 @ att_src2[0]
    w2e[:, 17] = W2 @ att_dst2[0]

    owner = dst // npc
    dloc = dst - owner * npc
    deg = np.zeros((ncores, npc), np.int64)
    for c in range(ncores):
        deg[c] = np.bincount(dloc[owner == c], minlength=npc)
    # processing order: degree-desc, stable
    pos_to_local = [np.argsort(-deg[c], kind="stable") for c in range(ncores)]
    pos_of_local = []
    for c in range(ncores):
        q = np.empty(npc, np.int64)
        q[pos_to_local[c]] = np.arange(npc)
        pos_of_local.append(q)
    # global processing position of every node (for layer-2 table rows)
    pos2_of = np.empty(N, np.int64)
    for c in range(ncores):
        pos2_of[c * npc:(c + 1) * npc] = c * npad + pos_of_local[c]

    per_core = []
    for c in range(ncores):
        m = owner == c
        s_c, d_c = src[m], dloc[m]
        epos = pos_of_local[c][d_c]
        o = np.argsort(epos, kind="stable")
        s_s, e_s = s_c[o], epos[o]
        counts = np.bincount(e_s, minlength=npad)
        starts = np.concatenate([[0], np.cumsum(counts)])
        l2src = pos2_of[s_s]
        per_core.append(dict(src1=s_s, src2=l2src, starts=starts))

    def layer_meta(key):
        nA = np.zeros((ncores, npad), np.int64)
        nB = np.zeros((ncores, npad), np.int64)
        for c in range(ncores):
            pc = per_core[c]
            isA = pc[key] < split
            pc[f"isA_{key}"] = isA
            st = pc["starts"]
            epos_all = np.repeat(np.arange(npad), np.diff(st))
            nA[c] = np.bincount(epos_all[isA], minlength=npad)
            nB[c] = np.bincount(epos_all[~isA], minlength=npad)
        DA, DB = [], []
        for t in range(tiles):
            sl = slice(t * P, (t + 1) * P)
            da = int(nA[:, sl].max()); db = int(nB[:, sl].max())
            da = (da + 3) // 4 * 4 if da else 0
            db = (db + 3) // 4 * 4 if db else 0
            assert da <= 63 and db <= 63, (da, db)
            DA.append(da); DB.append(db)
        return nA, nB, DA, DB

    nA1, nB1, DA1, DB1 = layer_meta("src1")
    nA2, nB2, DA2, DB2 = layer_meta("src2")

    def build_idx(key, nA, nB, DA, DB):
        blocks, offA, offB = [], [], []
        colpos = 0
        for t in range(tiles):
            offA.append(colpos)
            colpos += 8 * DA[t]
            offB.append(colpos)
            colpos += 8 * DB[t]
        per_core_img = []
        for c in range(ncores):
            pc = per_core[c]
            segs = []
            st = pc["starts"]; s = pc[key]; isA = pc[f"isA_{key}"]
            for t in range(tiles):
                for (D, wantA, base) in ((DA[t], True, 0), (DB[t], False, split)):
                    if D == 0:
                        continue
                    rect = np.zeros((P, D), np.int64)
                    for p in range(P):
                        pos = t * P + p
                        vals = s[st[pos]:st[pos + 1]]
                        mk = isA[st[pos]:st[pos + 1]]
                        v = vals[mk] if wantA else vals[~mk]
                        rect[p, :len(v)] = v - base
                    segs.append(_wrap16(rect))
            img = np.concatenate(segs, axis=1) if segs else np.zeros((P, 8), np.int16)
            per_core_img.append(img)
        return per_core_img, offA, offB, colpos

    idx1_imgs, offA1, offB1, C1 = build_idx("src1", nA1, nB1, DA1, DB1)
    idx2_imgs, offA2, offB2, C2 = build_idx("src2", nA2, nB2, DA2, DB2)

    # pad-count tensors m1 = [mA1|mB1|mA2|mB2] as [P, tiles] each
    m1_imgs, mo_imgs, idxo_imgs = [], [], []
    for c in range(ncores):
        def padcnt(nX, DX):
            a = np.zeros((P, tiles), np.float32)
            for t in range(tiles):
                a[:, t] = DX[t] - nX[c, t * P:(t + 1) * P]
            return a
        m1_imgs.append(np.concatenate(
            [padcnt(nA1, DA1), padcnt(nB1, DB1),
             padcnt(nA2, DA2), padcnt(nB2, DB2)], axis=1))
        # own rows
        ids = c * npc + pos_to_local[c]              # [npc]
        ids_pad = np.zeros(npad, np.int64)
        ids_pad[:npc] = ids
        valid = np.zeros(npad, bool)
        valid[:npc] = True
        isA = (ids_pad < split) & valid
        isB = (ids_pad >= split) & valid
        oA = np.where(isA, ids_pad, 0)
        oB = np.where(isB, ids_pad - split, 0)
        moA = (~isA).astype(np.float32)
        moB = (~isB).astype(np.float32)
        rectA = oA.reshape(tiles, P).T.copy()        # [P, tiles]
        rectB = oB.reshape(tiles, P).T.copy()
        idxo_imgs.append(np.concatenate([_wrap16(rectA), _wrap16(rectB)], axis=1))
        mo_imgs.append(np.concatenate(
            [moA.reshape(tiles, P).T, moB.reshape(tiles, P).T], axis=1).astype(np.float32))

    meta = dict(N=N, tiles=tiles, ncores=ncores, split=split,
                DA1=DA1, DB1=DB1, DA2=DA2, DB2=DB2,
                offA1=offA1, offB1=offB1, offA2=offA2, offB2=offB2)

    xTb = np.ascontiguousarray(x.T).astype(ml_dtypes.bfloat16)
    in_maps = []
    for c in range(ncores):
        in_maps.append({
            "xT": xTb,
            "w1e": w1e.astype(ml_dtypes.bfloat16),
            "w2e": w2e.astype(ml_dtypes.bfloat16),
            "b1i": np.tile(np.asarray(b1, np.float32)[None, :], (P, 1)),
            "b2i": np.tile(np.asarray(b2, np.float32)[None, :], (P, 1)),
            "idx1": idx1_imgs[c] if C1 else np.zeros((P, 8), np.int16),
            "idx2": idx2_imgs[c] if C2 else np.zeros((P, 8), np.int16),
            "idxo": idxo_imgs[c],
            "m1": m1_imgs[c],
            "mo": mo_imgs[c],
        })
    perm = []  # output row for (core, position)
    for c in range(ncores):
        perm.append(c * npc + pos_to_local[c])
    return meta, in_maps, perm


def _run(meta, in_maps, perm, ncores, tiles, npc, C):
    key = "prog"
    if key not in _CACHE:
        _CACHE[key] = _build_program(meta)
    nc = _CACHE[key]
    t0 = time.time()
    res = run_bass_kernel_spmd(nc, in_maps, list(range(ncores)))
    t1 = time.time()
    _CACHE["last_wall"] = t1 - t0
    N = meta["N"]
    out = np.zeros((N, C), np.float32)
    for c in range(ncores):
        o = res.results[c]["out"].reshape(P, tiles, 16)   # [p, t, c]
        o = o.transpose(1, 0, 2).reshape(tiles * P, 16)   # position-major
        out[perm[c]] = o[:npc]
    return out


def kernel(x, edge_index, W1, att_src1, att_dst1, b1, W2, att_src2, att_dst2, b2):
    x = np.asarray(x, np.float32)
    meta, in_maps, perm = _prep(
        x, edge_index, np.asarray(W1, np.float32),
        np.asarray(att_src1, np.float32), np.asarray(att_dst1, np.float32),
        np.asarray(b1, np.float32), np.asarray(W2, np.float32),
        np.asarray(att_src2, np.float32), np.asarray(att_dst2, np.float32),
        np.asarray(b2, np.float32),
        ncores=8, tiles=49, split=32768)
    return _run(meta, in_maps, perm, 8, 49, x.shape[0] // 8, 16)
